# revision 15
# baseline (speedup 1.0000x reference)
"""Local multi-headed attention (window +/-2) + residual + LayerNorm, Trainium2 Bass kernel.

Sharding: data-parallel over batch. B=8 batch elements -> one per NeuronCore (8 cores).
Each core computes the full sequence for its batch element; no collectives.

Per-core layout strategy:
  - x is loaded naturally [s,d] and PE-transposed to xT [d,s] (bf16) for the projections.
  - Q/K/V projections: W.T @ xT -> qT/kT/vT in [d_out, s] layout (heads on partitions,
    2 heads of 64 dims per 128-partition tile), bf16 weights, fp32 PSUM accumulate,
    PSUM->SBUF copy on ScalarE fused with the per-partition bias add (casting to bf16).
  - Scores: per window offset w, elementwise prod = qT * shift_w(kT) (DVE bf16), then one
    matmul with a block-ones [128,128] matrix which simultaneously reduces over the 64
    head dims AND broadcasts the per-head score back to all 64 partitions.
  - Zero-padded sequence ends reproduce the reference's pad semantics exactly:
    k_pad = 0 @ Wk + bk, arranged by filling halo columns with the bias.
  - Softmax over the 5 offsets: exp on ScalarE (scale=1/8 fused), denominator summed
    over taps on TensorE via a (blockones/64) stationary accumulating in PSUM,
    reciprocal_approx_fast on DVE, AV = sum_w exp_w * shift_w(vT) on DVE.
  - O-projection: attT blocks as stationary operand against Wo ->
    o in [s,d] layout; bias bo folded into the residual x on GpSimd.
  - Residual + LayerNorm in [s,d] layout with free-dim reductions; gamma/beta applied
    from partition-broadcast copies (built once via a K=1 matmul).
"""
import os
import sys
import numpy as np

B, S, D = 8, 4096, 768
HEADS = 12
DH = 64
W = 5          # window taps, offsets -2..2
CHUNK = 256    # sequence chunk per inner iteration
NCH = S // CHUNK
DT = D // 128  # 6 partition tiles of d
EPS = 1e-5
N_CORES = 8

_cache = {}


def _build():
    import bass_rust
    import concourse.bass as bass
    import concourse.tile as tile
    from concourse import bacc, mybir
    from concourse.masks import make_identity

    def taps_ap(base, n_taps):
        """Overlapping-taps view: base [128, S] -> [128, n_taps, S-...]
        rows at element offsets 0, 2, 4, ... (stride 2), each CHUNK long."""
        pstride = base.ap[0][0]
        return bass_rust.AP(base.tensor, base.offset,
                            [[pstride, 128], [2, n_taps], [1, CHUNK]])

    f32 = mybir.dt.float32
    f32r = mybir.dt.float32r
    bf16 = mybir.dt.bfloat16
    AF = mybir.ActivationFunctionType
    ALU = mybir.AluOpType

    nc = bacc.Bacc("TRN2", target_bir_lowering=False, debug=False,
                   num_devices=N_CORES)

    x_ap = nc.dram_tensor("x", [S, D], f32, kind="ExternalInput").ap()
    wq_ap = nc.dram_tensor("Wq", [D, D], f32, kind="ExternalInput").ap()
    bq_ap = nc.dram_tensor("bq", [D], f32, kind="ExternalInput").ap()
    wk_ap = nc.dram_tensor("Wk", [D, D], f32, kind="ExternalInput").ap()
    bk_ap = nc.dram_tensor("bk", [D], f32, kind="ExternalInput").ap()
    wv_ap = nc.dram_tensor("Wv", [D, D], f32, kind="ExternalInput").ap()
    bv_ap = nc.dram_tensor("bv", [D], f32, kind="ExternalInput").ap()
    wo_ap = nc.dram_tensor("Wo", [D, D], f32, kind="ExternalInput").ap()
    bo_ap = nc.dram_tensor("bo", [D], f32, kind="ExternalInput").ap()
    gamma_ap = nc.dram_tensor("gamma", [D], f32, kind="ExternalInput").ap()
    beta_ap = nc.dram_tensor("beta", [D], f32, kind="ExternalInput").ap()
    out_ap = nc.dram_tensor("out", [S, D], f32, kind="ExternalOutput").ap()

    with tile.TileContext(nc) as tc:
        # ---------------- persistent tiles ----------------
        with tc.tile_pool(name="persist", bufs=1) as pp:
            # weights: q/k/v in bf16 (score path tolerance), Wo in f32 (o path)
            wq_sb = pp.tile([128, DT, D], bf16, tag="wq")
            wk_sb = pp.tile([128, DT, D], bf16, tag="wk")
            wv_sb = pp.tile([128, DT, D], bf16, tag="wv")
            wo_sb = pp.tile([128, DT, D], bf16, tag="wo")
            bqT = pp.tile([128, DT], f32, tag="bqT")
            bkT = pp.tile([128, DT], f32, tag="bkT")
            bvT = pp.tile([128, DT], f32, tag="bvT")
            bo_sb = pp.tile([1, D], f32, tag="bo")
            g_sb = pp.tile([1, D], f32, tag="g")
            be_sb = pp.tile([1, D], f32, tag="be")
            ones_row = pp.tile([1, 128], f32, tag="ones")
            blockones = pp.tile([128, 128], bf16, tag="bones")
            blockones64 = pp.tile([128, 128], bf16, tag="bones64")
            ident = pp.tile([128, 128], f32, tag="ident")
            gb_bc = pp.tile([128, D], f32, tag="gbbc")
            be_bc = pp.tile([128, D], f32, tag="bebc")
            bo_bc = pp.tile([128, D], f32, tag="bobc")

            for w_ap, sb in ((wq_ap, wq_sb), (wk_ap, wk_sb), (wv_ap, wv_sb),
                             (wo_ap, wo_sb)):
                st = pp.tile([128, DT, D], f32, tag="wstage")
                nc.sync.dma_start(st[:], w_ap.rearrange("(kt p) n -> p kt n", p=128))
                nc.vector.tensor_copy(sb[:], st[:])
            nc.sync.dma_start(bqT[:], bq_ap.rearrange("(t p) -> p t", p=128))
            nc.sync.dma_start(bkT[:], bk_ap.rearrange("(t p) -> p t", p=128))
            nc.sync.dma_start(bvT[:], bv_ap.rearrange("(t p) -> p t", p=128))
            nc.sync.dma_start(bo_sb[:], bo_ap[:])
            nc.sync.dma_start(g_sb[:], gamma_ap[:])
            nc.sync.dma_start(be_sb[:], beta_ap[:])

            nc.vector.memset(ones_row[:], 1.0)
            nc.vector.memset(blockones[:], 0.0)
            nc.vector.memset(blockones[0:64, 0:64], 1.0)
            nc.vector.memset(blockones[64:128, 64:128], 1.0)
            nc.vector.memset(blockones64[:], 0.0)
            nc.vector.memset(blockones64[0:64, 0:64], 1.0 / 64.0)
            nc.vector.memset(blockones64[64:128, 64:128], 1.0 / 64.0)
            make_identity(nc, ident[:])

            # broadcast gamma/beta/bo across partitions via K=1 matmul
            with tc.tile_pool(name="initps", bufs=1, space="PSUM") as initps:
                for src, dst in ((g_sb, gb_bc), (be_sb, be_bc), (bo_sb, bo_bc)):
                    t = initps.tile([128, D], f32, tag="gbps")
                    nc.tensor.matmul(t[:, 0:512], ones_row[:], src[:, 0:512])
                    nc.tensor.matmul(t[:, 512:D], ones_row[:], src[:, 512:D])
                    nc.vector.tensor_copy(dst[:], t[:])

            # ---------------- working pools ----------------
            with tc.tile_pool(name="ppsum", bufs=2, space="PSUM") as ppsum, \
                 tc.tile_pool(name="spsum", bufs=1, space="PSUM") as spsum, \
                 tc.tile_pool(name="dnpsum", bufs=1, space="PSUM") as dnpsum, \
                 tc.tile_pool(name="opsum", bufs=1, space="PSUM") as opsum, \
                 tc.tile_pool(name="xpool", bufs=3) as xpool, \
                 tc.tile_pool(name="xtpool", bufs=2) as xtpool, \
                 tc.tile_pool(name="qpool", bufs=2) as qpool, \
                 tc.tile_pool(name="kvpool", bufs=4) as kvpool, \
                 tc.tile_pool(name="atpool", bufs=2) as atpool, \
                 tc.tile_pool(name="appool", bufs=2) as appool, \
                 tc.tile_pool(name="dnpool", bufs=2) as dnpool, \
                 tc.tile_pool(name="ypool", bufs=2) as ypool, \
                 tc.tile_pool(name="stpool", bufs=2) as stpool:

                kc_tiles = [None] * NCH
                vc_tiles = [None] * NCH

                def project(c):
                    """projections for chunk c -> qT (bf16) and kc/vc center cols."""
                    s0 = c * CHUNK
                    # load x naturally, 2 s-subtiles of 128
                    x_sb = xpool.tile([128, 2, D], f32, tag="x")
                    nc.sync.dma_start(
                        x_sb[:], x_ap[s0:s0 + CHUNK, :].rearrange(
                            "(st p) d -> p st d", p=128))
                    # transpose to xT bf16 [128, DT, CHUNK]
                    xT = xtpool.tile([128, DT, CHUNK], bf16, tag="xT")
                    for dt in range(DT):
                        tp = ppsum.tile([128, CHUNK], f32, tag="proj")
                        for st in range(2):
                            nc.tensor.transpose(
                                tp[:, st * 128:(st + 1) * 128],
                                x_sb[:, st, dt * 128:(dt + 1) * 128], ident[:])
                        nc.scalar.copy(xT[:, dt, :], tp[:])
                    # fold bo into the residual on GpSimd AFTER the transposes
                    # consumed x (removes the PE bias matmuls from the O-proj)
                    for st in range(2):
                        nc.gpsimd.tensor_tensor(x_sb[:, st, :], x_sb[:, st, :],
                                                bo_bc[:], ALU.add)

                    qT = qpool.tile([128, DT, CHUNK], bf16, tag="qT")
                    kc = kvpool.tile([128, DT, CHUNK + 4], bf16, tag="kc")
                    vc = kvpool.tile([128, DT, CHUNK + 4], bf16, tag="vc")
                    kc_tiles[c] = kc
                    vc_tiles[c] = vc
                    for (wsb, bT, dst, off) in ((wq_sb, bqT, qT, None),
                                                (wk_sb, bkT, kc, 2),
                                                (wv_sb, bvT, vc, 2)):
                        for dt in range(DT):
                            ps = ppsum.tile([128, CHUNK], f32, tag="proj")
                            for kt in range(DT):
                                nc.tensor.matmul(
                                    ps[:],
                                    wsb[:, kt, dt * 128:(dt + 1) * 128],
                                    xT[:, kt, :],
                                    start=(kt == 0), stop=(kt == DT - 1))
                            dslice = dst[:, dt, :] if off is None \
                                else dst[:, dt, 2:2 + CHUNK]
                            nc.scalar.activation(dslice, ps[:], AF.Identity,
                                                 bias=bT[:, dt:dt + 1])
                    # halo fills
                    if c > 0:
                        # left halo of c <- tail of c-1 center; and
                        # right halo of c-1 <- head of c center
                        for big_prev, big_cur in ((kc_tiles[c - 1], kc),
                                                  (vc_tiles[c - 1], vc)):
                            nc.gpsimd.tensor_copy(big_cur[:, :, 0:2],
                                                  big_prev[:, :, CHUNK:CHUNK + 2])
                            nc.gpsimd.tensor_copy(big_prev[:, :, CHUNK + 2:CHUNK + 4],
                                                  big_cur[:, :, 2:4])
                    if c == 0:
                        for big, bT in ((kc, bkT), (vc, bvT)):
                            for dt in range(DT):
                                nc.vector.memset(big[:, dt, 0:2], 0.0)
                                nc.scalar.activation(big[:, dt, 0:2],
                                                     big[:, dt, 0:2],
                                                     AF.Identity,
                                                     bias=bT[:, dt:dt + 1])
                    if c == NCH - 1:
                        for big, bT in ((kc, bkT), (vc, bvT)):
                            for dt in range(DT):
                                nc.vector.memset(big[:, dt, CHUNK + 2:CHUNK + 4], 0.0)
                                nc.scalar.activation(big[:, dt, CHUNK + 2:CHUNK + 4],
                                                     big[:, dt, CHUNK + 2:CHUNK + 4],
                                                     AF.Identity,
                                                     bias=bT[:, dt:dt + 1])
                    return x_sb, qT

                def attention(c, x_sb, qT):
                    """scores/softmax/AV/O-proj/LN for chunk c (projections done)."""
                    s0 = c * CHUNK
                    kc, vc = kc_tiles[c], vc_tiles[c]
                    att = atpool.tile([128, DT, CHUNK], bf16, tag="att")
                    for dt in range(DT):
                        # products: merged even taps (2x bf16 mode) + merged
                        # odd taps (1x, misaligned) via overlapping-stride APs
                        prod = appool.tile([128, W, CHUNK], bf16, tag="prod")
                        q_bc3 = qT[:, dt, :].unsqueeze(1).broadcast_to(
                            [128, 3, CHUNK])
                        q_bc2 = qT[:, dt, :].unsqueeze(1).broadcast_to(
                            [128, 2, CHUNK])
                        nc.vector.tensor_tensor(
                            prod[:, 0:W:2, :], q_bc3,
                            taps_ap(kc[:, dt, 0:CHUNK], 3), ALU.mult)
                        nc.vector.tensor_tensor(
                            prod[:, 1:W:2, :], q_bc2,
                            taps_ap(kc[:, dt, 1:1 + CHUNK], 2), ALU.mult)
                        # scores + head-reduce + broadcast in one matmul per tap
                        sc = spsum.tile([128, W, CHUNK], f32, tag="scores")
                        for w in range(W):
                            nc.tensor.matmul(sc[:, w, :], blockones[:],
                                             prod[:, w, :])
                        # exp with fused 1/sqrt(dh) scale
                        ex = appool.tile([128, W, CHUNK], bf16, tag="exp")
                        nc.scalar.activation(ex[:], sc[:], AF.Exp, scale=0.125)
                        # denominator: sum the 5 taps on TensorE via the
                        # (blockones/64) stationary (rows within a head block
                        # are identical, so the 64-row mean reproduces each
                        # tap exactly while PSUM accumulates over taps).
                        dn_ps = dnpsum.tile([128, CHUNK], f32, tag="dnps")
                        for w in range(W):
                            nc.tensor.matmul(dn_ps[:], blockones64[:],
                                             ex[:, w, :],
                                             start=(w == 0), stop=(w == W - 1))
                        rinv = dnpool.tile([128, CHUNK], f32, tag="rinv")
                        nc.vector.reciprocal_approx_fast(rinv[:], dn_ps[:])
                        # AV: avp_w = exp_w * v_tap_w (merged even/odd), then
                        # pairwise tap-sum tree
                        avp = appool.tile([128, W, CHUNK], bf16, tag="avp")
                        nc.vector.tensor_tensor(
                            avp[:, 0:W:2, :], ex[:, 0:W:2, :],
                            taps_ap(vc[:, dt, 0:CHUNK], 3), ALU.mult)
                        nc.vector.tensor_tensor(
                            avp[:, 1:W:2, :], ex[:, 1:W:2, :],
                            taps_ap(vc[:, dt, 1:1 + CHUNK], 2), ALU.mult)
                        pair = dnpool.tile([128, 2, CHUNK], bf16, tag="pair")
                        nc.vector.tensor_tensor(pair[:], avp[:, 0:2, :],
                                                avp[:, 2:4, :], ALU.add)
                        asum = dnpool.tile([128, CHUNK], bf16, tag="asum")
                        nc.vector.tensor_tensor(asum[:], pair[:, 0, :],
                                                pair[:, 1, :], ALU.add)
                        nc.vector.tensor_tensor(asum[:], asum[:], avp[:, 4, :],
                                                ALU.add)
                        nc.vector.tensor_tensor(att[:, dt, :], asum[:], rinv[:],
                                                ALU.mult)

                    # O-projection + bias + residual + LayerNorm per s-tile
                    for st in range(2):
                        op = opsum.tile([128, D], f32, tag="o")
                        for dt in range(DT):
                            a_blk = att[:, dt, st * 128:(st + 1) * 128]
                            nc.tensor.matmul(op[:, 0:512], a_blk,
                                             wo_sb[:, dt, 0:512],
                                             start=(dt == 0), stop=(dt == DT - 1))
                            nc.tensor.matmul(op[:, 512:D], a_blk,
                                             wo_sb[:, dt, 512:D],
                                             start=(dt == 0), stop=(dt == DT - 1))
                        ypre = ypool.tile([128, D], f32, tag="ypre")
                        nc.vector.tensor_tensor(ypre[:], op[:], x_sb[:, st, :],
                                                ALU.add)
                        # LayerNorm stats
                        stats = stpool.tile([128, 8], f32, tag="stats")
                        dump = stpool.tile([128, D], bf16, tag="dump")
                        nc.vector.tensor_reduce(stats[:, 0:1], ypre[:],
                                                axis=mybir.AxisListType.X,
                                                op=ALU.add)
                        nc.scalar.activation(dump[:], ypre[:], AF.Square,
                                             accum_out=stats[:, 1:2])
                        # var = (sumsq - sum^2/768)/768 ; rstd = 1/sqrt(var+eps)
                        nc.vector.tensor_tensor(stats[:, 2:3], stats[:, 0:1],
                                                stats[:, 0:1], ALU.mult)
                        nc.vector.tensor_scalar_mul(stats[:, 2:3], stats[:, 2:3],
                                                    -1.0 / D)
                        nc.vector.tensor_tensor(stats[:, 2:3], stats[:, 2:3],
                                                stats[:, 1:2], ALU.add)
                        nc.vector.tensor_scalar(stats[:, 3:4], stats[:, 2:3],
                                                1.0 / D, EPS, ALU.mult, ALU.add)
                        # rstd = exp(-0.5*ln(var+eps)); Ln/Exp share one ACT
                        # table set (Sqrt does not -> avoids table reloads)
                        nc.scalar.activation(stats[:, 4:5], stats[:, 3:4], AF.Ln)
                        nc.scalar.activation(stats[:, 5:6], stats[:, 4:5],
                                             AF.Exp, scale=-0.5)
                        # negmurstd = -sum/D * rstd
                        nc.vector.tensor_tensor(stats[:, 6:7], stats[:, 0:1],
                                                stats[:, 5:6], ALU.mult)
                        nc.vector.tensor_scalar_mul(stats[:, 6:7], stats[:, 6:7],
                                                    -1.0 / D)
                        y1 = ypool.tile([128, D], f32, tag="y1")
                        nc.scalar.activation(y1[:], ypre[:], AF.Identity,
                                             bias=stats[:, 6:7],
                                             scale=stats[:, 5:6])
                        y2 = ypool.tile([128, D], f32, tag="y2")
                        nc.gpsimd.tensor_tensor(y2[:], y1[:], gb_bc[:], ALU.mult)
                        nc.gpsimd.tensor_tensor(y2[:], y2[:], be_bc[:], ALU.add)
                        nc.sync.dma_start(
                            out_ap[s0 + st * 128: s0 + (st + 1) * 128, :], y2[:])

                # run projections one chunk ahead of attention (right halo dep)
                pend = None
                for c in range(NCH):
                    cur = project(c)
                    if pend is not None:
                        attention(c - 1, *pend)
                    pend = cur
                attention(NCH - 1, *pend)

    nc.compile()
    return nc


def kernel(**inputs):
    if "nc" not in _cache:
        _cache["nc"] = _build()
    nc = _cache["nc"]
    from concourse.bass_utils import run_bass_kernel_spmd

    names = ["Wq", "bq", "Wk", "bk", "Wv", "bv", "Wo", "bo", "gamma", "beta"]
    shared = {n: np.ascontiguousarray(np.asarray(inputs[n], dtype=np.float32))
              for n in names}
    x = np.asarray(inputs["x"], dtype=np.float32)
    in_maps = [dict(shared, x=np.ascontiguousarray(x[b])) for b in range(N_CORES)]
    res = run_bass_kernel_spmd(nc, in_maps, core_ids=list(range(N_CORES)))
    out = np.stack([res.results[i]["out"] for i in range(N_CORES)], axis=0)
    return out.astype(np.float32)



# revision 22
# speedup vs baseline: 1.1025x; 1.1025x over previous
"""Local multi-headed attention (window +/-2) + residual + LayerNorm, Trainium2 Bass kernel.

Sharding: data-parallel over batch. B=8 batch elements -> one per NeuronCore (8 cores).
Each core computes the full sequence for its batch element; no collectives.

Per-core layout strategy:
  - x is loaded naturally [s,d] and PE-transposed to xT [d,s] (bf16) for the projections.
  - Q/K/V projections: W.T @ xT -> qT/kT/vT in [d_out, s] layout (heads on partitions,
    2 heads of 64 dims per 128-partition tile), bf16 weights, fp32 PSUM accumulate,
    PSUM->SBUF copy on ScalarE fused with the per-partition bias add (casting to bf16).
  - Scores: per window offset w, elementwise prod = qT * shift_w(kT) (DVE bf16), then one
    matmul with a block-ones [128,128] matrix which simultaneously reduces over the 64
    head dims AND broadcasts the per-head score back to all 64 partitions.
  - Zero-padded sequence ends reproduce the reference's pad semantics exactly:
    k_pad = 0 @ Wk + bk, arranged by filling halo columns with the bias.
  - Softmax over the 5 offsets: exp on ScalarE (scale=1/8 fused), denominator summed
    over taps on TensorE via a (blockones/64) stationary accumulating in PSUM,
    reciprocal_approx_fast on DVE, AV = sum_w exp_w * shift_w(vT) on DVE.
  - O-projection: attT blocks as stationary operand against Wo ->
    o in [s,d] layout; bias bo folded into the residual x on GpSimd.
  - Residual + LayerNorm in [s,d] layout with free-dim reductions; gamma/beta applied
    from partition-broadcast copies (built once via a K=1 matmul).
"""
import os
import sys
import numpy as np

B, S, D = 8, 4096, 768
HEADS = 12
DH = 64
W = 5          # window taps, offsets -2..2
CHUNK = 256    # sequence chunk per inner iteration
NCH = S // CHUNK
DT = D // 128  # 6 partition tiles of d
EPS = 1e-5
N_CORES = 8

_cache = {}


def _build():
    import bass_rust
    import concourse.bass as bass
    import concourse.tile as tile
    from concourse import bacc, mybir
    from concourse.masks import make_identity

    def taps_ap(base, n_taps):
        """Overlapping-taps view: base [128, S] -> [128, n_taps, S-...]
        rows at element offsets 0, 2, 4, ... (stride 2), each CHUNK long."""
        pstride = base.ap[0][0]
        return bass_rust.AP(base.tensor, base.offset,
                            [[pstride, 128], [2, n_taps], [1, CHUNK]])

    f32 = mybir.dt.float32
    f32r = mybir.dt.float32r
    bf16 = mybir.dt.bfloat16
    AF = mybir.ActivationFunctionType
    ALU = mybir.AluOpType

    nc = bacc.Bacc("TRN2", target_bir_lowering=False, debug=False,
                   num_devices=N_CORES)

    x_ap = nc.dram_tensor("x", [S, D], f32, kind="ExternalInput").ap()
    wq_ap = nc.dram_tensor("Wq", [D, D], f32, kind="ExternalInput").ap()
    bq_ap = nc.dram_tensor("bq", [D], f32, kind="ExternalInput").ap()
    wk_ap = nc.dram_tensor("Wk", [D, D], f32, kind="ExternalInput").ap()
    bk_ap = nc.dram_tensor("bk", [D], f32, kind="ExternalInput").ap()
    wv_ap = nc.dram_tensor("Wv", [D, D], f32, kind="ExternalInput").ap()
    bv_ap = nc.dram_tensor("bv", [D], f32, kind="ExternalInput").ap()
    wo_ap = nc.dram_tensor("Wo", [D, D], f32, kind="ExternalInput").ap()
    bo_ap = nc.dram_tensor("bo", [D], f32, kind="ExternalInput").ap()
    gamma_ap = nc.dram_tensor("gamma", [D], f32, kind="ExternalInput").ap()
    beta_ap = nc.dram_tensor("beta", [D], f32, kind="ExternalInput").ap()
    out_ap = nc.dram_tensor("out", [S, D], f32, kind="ExternalOutput").ap()

    with tile.TileContext(nc) as tc:
        # ---------------- persistent tiles ----------------
        with tc.tile_pool(name="persist", bufs=1) as pp:
            # weights: q/k/v in bf16 (score path tolerance), Wo in f32 (o path)
            wq_sb = pp.tile([128, DT, D], bf16, tag="wq")
            wk_sb = pp.tile([128, DT, D], bf16, tag="wk")
            wv_sb = pp.tile([128, DT, D], bf16, tag="wv")
            wo_sb = pp.tile([128, DT, D], bf16, tag="wo")
            bqT = pp.tile([128, DT], f32, tag="bqT")
            bkT = pp.tile([128, DT], f32, tag="bkT")
            bvT = pp.tile([128, DT], f32, tag="bvT")
            bo_sb = pp.tile([1, D], f32, tag="bo")
            g_sb = pp.tile([1, D], f32, tag="g")
            be_sb = pp.tile([1, D], f32, tag="be")
            ones_row = pp.tile([1, 128], f32, tag="ones")
            blockones = pp.tile([128, 128], bf16, tag="bones")
            blockones64 = pp.tile([128, 128], bf16, tag="bones64")
            ident = pp.tile([128, 128], f32, tag="ident")
            gb_bc = pp.tile([128, D], f32, tag="gbbc")
            be_bc = pp.tile([128, D], f32, tag="bebc")
            bo_bc = pp.tile([128, D], f32, tag="bobc")

            for w_ap, sb in ((wq_ap, wq_sb), (wk_ap, wk_sb), (wv_ap, wv_sb),
                             (wo_ap, wo_sb)):
                st = pp.tile([128, DT, D], f32, tag="wstage")
                nc.sync.dma_start(st[:], w_ap.rearrange("(kt p) n -> p kt n", p=128))
                nc.vector.tensor_copy(sb[:], st[:])
            nc.sync.dma_start(bqT[:], bq_ap.rearrange("(t p) -> p t", p=128))
            nc.sync.dma_start(bkT[:], bk_ap.rearrange("(t p) -> p t", p=128))
            nc.sync.dma_start(bvT[:], bv_ap.rearrange("(t p) -> p t", p=128))
            nc.sync.dma_start(bo_sb[:], bo_ap[:])
            nc.sync.dma_start(g_sb[:], gamma_ap[:])
            nc.sync.dma_start(be_sb[:], beta_ap[:])

            nc.vector.memset(ones_row[:], 1.0)
            nc.vector.memset(blockones[:], 0.0)
            nc.vector.memset(blockones[0:64, 0:64], 1.0)
            nc.vector.memset(blockones[64:128, 64:128], 1.0)
            nc.vector.memset(blockones64[:], 0.0)
            nc.vector.memset(blockones64[0:64, 0:64], 1.0 / 64.0)
            nc.vector.memset(blockones64[64:128, 64:128], 1.0 / 64.0)
            make_identity(nc, ident[:])

            # fold bv into bo: att = sum_w p_w (v0_w + bv) = sum_w p_w v0_w + bv
            # (softmax weights sum to 1), so (att+bv)@Wo = att@Wo + bv@Wo.
            # bo_total = bo + bv @ Wo, then broadcast across partitions.
            bvT_bf = pp.tile([128, DT], bf16, tag="bvbf")
            nc.vector.tensor_copy(bvT_bf[:], bvT[:])
            with tc.tile_pool(name="initps", bufs=1, space="PSUM") as initps:
                bvwo = initps.tile([1, D], f32, tag="bvwo")
                for kt in range(DT):
                    nc.tensor.matmul(bvwo[:, 0:512], bvT_bf[:, kt:kt + 1],
                                     wo_sb[:, kt, 0:512],
                                     start=(kt == 0), stop=(kt == DT - 1))
                for kt in range(DT):
                    nc.tensor.matmul(bvwo[:, 512:D], bvT_bf[:, kt:kt + 1],
                                     wo_sb[:, kt, 512:D],
                                     start=(kt == 0), stop=(kt == DT - 1))
                nc.vector.tensor_tensor(bo_sb[:], bo_sb[:], bvwo[:], ALU.add)
                for src, dst in ((g_sb, gb_bc), (be_sb, be_bc), (bo_sb, bo_bc)):
                    t = initps.tile([128, D], f32, tag="gbps")
                    nc.tensor.matmul(t[:, 0:512], ones_row[:], src[:, 0:512])
                    nc.tensor.matmul(t[:, 512:D], ones_row[:], src[:, 512:D])
                    nc.vector.tensor_copy(dst[:], t[:])

            # ---------------- working pools ----------------
            with tc.tile_pool(name="ppsum", bufs=2, space="PSUM") as ppsum, \
                 tc.tile_pool(name="spsum", bufs=1, space="PSUM") as spsum, \
                 tc.tile_pool(name="dnpsum", bufs=1, space="PSUM") as dnpsum, \
                 tc.tile_pool(name="opsum", bufs=1, space="PSUM") as opsum, \
                 tc.tile_pool(name="xpool", bufs=3) as xpool, \
                 tc.tile_pool(name="xtpool", bufs=2) as xtpool, \
                 tc.tile_pool(name="qpool", bufs=2) as qpool, \
                 tc.tile_pool(name="kvpool", bufs=4) as kvpool, \
                 tc.tile_pool(name="kvshpool", bufs=4) as kvshpool, \
                 tc.tile_pool(name="atpool", bufs=2) as atpool, \
                 tc.tile_pool(name="appool", bufs=2) as appool, \
                 tc.tile_pool(name="dnpool", bufs=2) as dnpool, \
                 tc.tile_pool(name="ypool", bufs=2) as ypool, \
                 tc.tile_pool(name="stpool", bufs=2) as stpool:

                kc_tiles = [None] * NCH
                vc_tiles = [None] * NCH
                ksh_tiles = [None] * NCH
                vsh_tiles = [None] * NCH

                def project(c):
                    """projections for chunk c -> qT (bf16) and kc/vc center cols."""
                    s0 = c * CHUNK
                    # load x naturally, 2 s-subtiles of 128
                    x_sb = xpool.tile([128, 2, D], f32, tag="x")
                    nc.sync.dma_start(
                        x_sb[:], x_ap[s0:s0 + CHUNK, :].rearrange(
                            "(st p) d -> p st d", p=128))
                    # transpose to xT bf16 [128, DT, CHUNK]; 2 dt-planes share
                    # one PSUM bank, drained by a single paired copy
                    xT = xtpool.tile([128, DT, CHUNK], bf16, tag="xT")
                    for dtp in range(DT // 2):
                        tp = ppsum.tile([128, 2, CHUNK], f32, tag="proj")
                        for pl in range(2):
                            dt = 2 * dtp + pl
                            for st in range(2):
                                nc.tensor.transpose(
                                    tp[:, pl, st * 128:(st + 1) * 128],
                                    x_sb[:, st, dt * 128:(dt + 1) * 128],
                                    ident[:])
                        nc.scalar.copy(xT[:, 2 * dtp:2 * dtp + 2, :], tp[:])
                    # fold bo_total into the residual on GpSimd AFTER the
                    # transposes consumed x
                    for st in range(2):
                        nc.gpsimd.tensor_tensor(x_sb[:, st, :], x_sb[:, st, :],
                                                bo_bc[:], ALU.add)

                    # projections. K/V are BIAS-FREE: the k-bias shifts all 5
                    # scores of a token equally (softmax-invariant, pad slots
                    # score 0 = the shifted pad score), and the v-bias was
                    # folded into bo via bv@Wo since softmax weights sum to 1.
                    qT = qpool.tile([128, DT, CHUNK], bf16, tag="qT")
                    kc = kvpool.tile([128, DT, CHUNK + 4], bf16, tag="kc")
                    vc = kvpool.tile([128, DT, CHUNK + 4], bf16, tag="vc")
                    ksh = kvshpool.tile([128, DT, CHUNK + 2], bf16, tag="ksh")
                    vsh = kvshpool.tile([128, DT, CHUNK + 2], bf16, tag="vsh")
                    kc_tiles[c] = kc
                    vc_tiles[c] = vc
                    ksh_tiles[c] = ksh
                    vsh_tiles[c] = vsh
                    for (wsb, bT, dst) in ((wq_sb, bqT, qT),
                                           (wk_sb, None, kc),
                                           (wv_sb, None, vc)):
                        for dtp in range(DT // 2):
                            ps = ppsum.tile([128, 2, CHUNK], f32, tag="proj")
                            for pl in range(2):
                                dt = 2 * dtp + pl
                                for kt in range(DT):
                                    nc.tensor.matmul(
                                        ps[:, pl, :],
                                        wsb[:, kt, dt * 128:(dt + 1) * 128],
                                        xT[:, kt, :],
                                        start=(kt == 0), stop=(kt == DT - 1))
                            if bT is not None:
                                for pl in range(2):
                                    dt = 2 * dtp + pl
                                    nc.scalar.activation(
                                        dst[:, dt, :], ps[:, pl, :],
                                        AF.Identity, bias=bT[:, dt:dt + 1])
                            else:
                                nc.scalar.copy(
                                    dst[:, 2 * dtp:2 * dtp + 2, 2:2 + CHUNK],
                                    ps[:])
                    # halo fills (bias-free k/v values are chunk-consistent)
                    if c > 0:
                        # left halo of c <- tail of c-1 center; and
                        # right halo of c-1 <- head of c center
                        for big_prev, big_cur in ((kc_tiles[c - 1], kc),
                                                  (vc_tiles[c - 1], vc)):
                            nc.gpsimd.tensor_copy(big_cur[:, :, 0:2],
                                                  big_prev[:, :, CHUNK:CHUNK + 2])
                            nc.gpsimd.tensor_copy(big_prev[:, :, CHUNK + 2:CHUNK + 4],
                                                  big_cur[:, :, 2:4])
                        # c-1 tiles are now final: build its odd-tap shifted
                        # copies (element +1) so odd taps hit 2x DVE mode
                        nc.sync.dma_start(ksh_tiles[c - 1][:],
                                          kc_tiles[c - 1][:, :, 1:3 + CHUNK])
                        nc.sync.dma_start(vsh_tiles[c - 1][:],
                                          vc_tiles[c - 1][:, :, 1:3 + CHUNK])
                    if c == 0:
                        for big in (kc, vc):
                            nc.vector.memset(big[:, :, 0:2], 0.0)
                    if c == NCH - 1:
                        for big in (kc, vc):
                            nc.vector.memset(big[:, :, CHUNK + 2:CHUNK + 4], 0.0)
                        nc.sync.dma_start(ksh[:], kc[:, :, 1:3 + CHUNK])
                        nc.sync.dma_start(vsh[:], vc[:, :, 1:3 + CHUNK])
                    return x_sb, qT

                def attention(c, x_sb, qT):
                    """scores/softmax/AV/O-proj/LN for chunk c (projections done)."""
                    s0 = c * CHUNK
                    kc, vc = kc_tiles[c], vc_tiles[c]
                    ksh, vsh = ksh_tiles[c], vsh_tiles[c]
                    att = atpool.tile([128, DT, CHUNK], bf16, tag="att")
                    for dt in range(DT):
                        # products: merged even taps from kc, merged odd taps
                        # from the +1-shifted copy (both 2x bf16 aligned)
                        prod = appool.tile([128, W, CHUNK], bf16, tag="prod")
                        q_bc3 = qT[:, dt, :].unsqueeze(1).broadcast_to(
                            [128, 3, CHUNK])
                        q_bc2 = qT[:, dt, :].unsqueeze(1).broadcast_to(
                            [128, 2, CHUNK])
                        nc.vector.tensor_tensor(
                            prod[:, 0:W:2, :], q_bc3,
                            taps_ap(kc[:, dt, 0:CHUNK], 3), ALU.mult)
                        nc.vector.tensor_tensor(
                            prod[:, 1:W:2, :], q_bc2,
                            taps_ap(ksh[:, dt, 0:CHUNK], 2), ALU.mult)
                        # scores + head-reduce + broadcast in one matmul per tap
                        sc = spsum.tile([128, W, CHUNK], f32, tag="scores")
                        for w in range(W):
                            nc.tensor.matmul(sc[:, w, :], blockones[:],
                                             prod[:, w, :])
                        # exp with fused 1/sqrt(dh) scale, split in two so the
                        # first pair drains while taps 2-4 still matmul
                        ex = appool.tile([128, W, CHUNK], bf16, tag="exp")
                        nc.scalar.activation(ex[:, 0:2, :], sc[:, 0:2, :],
                                             AF.Exp, scale=0.125)
                        nc.scalar.activation(ex[:, 2:W, :], sc[:, 2:W, :],
                                             AF.Exp, scale=0.125)
                        # denominator: sum the 5 taps on TensorE via the
                        # (blockones/64) stationary (rows within a head block
                        # are identical, so the 64-row mean reproduces each
                        # tap exactly while PSUM accumulates over taps).
                        dn_ps = dnpsum.tile([128, CHUNK], f32, tag="dnps")
                        for w in range(W):
                            nc.tensor.matmul(dn_ps[:], blockones64[:],
                                             ex[:, w, :],
                                             start=(w == 0), stop=(w == W - 1))
                        rinv = dnpool.tile([128, CHUNK], f32, tag="rinv")
                        nc.vector.reciprocal_approx_fast(rinv[:], dn_ps[:])
                        # AV: avp_w = exp_w * v_tap_w (merged even/odd), then
                        # pairwise tap-sum tree
                        avp = appool.tile([128, W, CHUNK], bf16, tag="avp")
                        nc.vector.tensor_tensor(
                            avp[:, 0:W:2, :], ex[:, 0:W:2, :],
                            taps_ap(vc[:, dt, 0:CHUNK], 3), ALU.mult)
                        nc.vector.tensor_tensor(
                            avp[:, 1:W:2, :], ex[:, 1:W:2, :],
                            taps_ap(vsh[:, dt, 0:CHUNK], 2), ALU.mult)
                        pair = dnpool.tile([128, 2, CHUNK], bf16, tag="pair")
                        nc.vector.tensor_tensor(pair[:], avp[:, 0:2, :],
                                                avp[:, 2:4, :], ALU.add)
                        asum = dnpool.tile([128, CHUNK], bf16, tag="asum")
                        nc.vector.tensor_tensor(asum[:], pair[:, 0, :],
                                                pair[:, 1, :], ALU.add)
                        nc.vector.tensor_tensor(asum[:], asum[:], avp[:, 4, :],
                                                ALU.add)
                        nc.vector.tensor_tensor(att[:, dt, :], asum[:], rinv[:],
                                                ALU.mult)

                    # O-projection + bias + residual + LayerNorm per s-tile
                    for st in range(2):
                        op = opsum.tile([128, D], f32, tag="o")
                        for dt in range(DT):
                            a_blk = att[:, dt, st * 128:(st + 1) * 128]
                            nc.tensor.matmul(op[:, 0:512], a_blk,
                                             wo_sb[:, dt, 0:512],
                                             start=(dt == 0), stop=(dt == DT - 1))
                            nc.tensor.matmul(op[:, 512:D], a_blk,
                                             wo_sb[:, dt, 512:D],
                                             start=(dt == 0), stop=(dt == DT - 1))
                        ypre = ypool.tile([128, D], f32, tag="ypre")
                        nc.vector.tensor_tensor(ypre[:], op[:], x_sb[:, st, :],
                                                ALU.add)
                        # LayerNorm stats
                        stats = stpool.tile([128, 8], f32, tag="stats")
                        dump = stpool.tile([128, D], bf16, tag="dump")
                        nc.vector.tensor_reduce(stats[:, 0:1], ypre[:],
                                                axis=mybir.AxisListType.X,
                                                op=ALU.add)
                        nc.scalar.activation(dump[:], ypre[:], AF.Square,
                                             accum_out=stats[:, 1:2])
                        # var = (sumsq - sum^2/768)/768 ; rstd = 1/sqrt(var+eps)
                        nc.vector.tensor_tensor(stats[:, 2:3], stats[:, 0:1],
                                                stats[:, 0:1], ALU.mult)
                        nc.vector.tensor_scalar_mul(stats[:, 2:3], stats[:, 2:3],
                                                    -1.0 / D)
                        nc.vector.tensor_tensor(stats[:, 2:3], stats[:, 2:3],
                                                stats[:, 1:2], ALU.add)
                        nc.vector.tensor_scalar(stats[:, 3:4], stats[:, 2:3],
                                                1.0 / D, EPS, ALU.mult, ALU.add)
                        nc.scalar.sqrt(stats[:, 4:5], stats[:, 3:4])
                        nc.vector.reciprocal(stats[:, 5:6], stats[:, 4:5])
                        # negmurstd = -sum/D * rstd
                        nc.vector.tensor_tensor(stats[:, 6:7], stats[:, 0:1],
                                                stats[:, 5:6], ALU.mult)
                        nc.vector.tensor_scalar_mul(stats[:, 6:7], stats[:, 6:7],
                                                    -1.0 / D)
                        y1 = ypool.tile([128, D], f32, tag="y1")
                        nc.scalar.activation(y1[:], ypre[:], AF.Identity,
                                             bias=stats[:, 6:7],
                                             scale=stats[:, 5:6])
                        y2 = ypool.tile([128, D], f32, tag="y2")
                        nc.gpsimd.tensor_tensor(y2[:], y1[:], gb_bc[:], ALU.mult)
                        nc.gpsimd.tensor_tensor(y2[:], y2[:], be_bc[:], ALU.add)
                        nc.sync.dma_start(
                            out_ap[s0 + st * 128: s0 + (st + 1) * 128, :], y2[:])

                # run projections one chunk ahead of attention (right halo dep)
                pend = None
                for c in range(NCH):
                    cur = project(c)
                    if pend is not None:
                        attention(c - 1, *pend)
                    pend = cur
                attention(NCH - 1, *pend)

    nc.compile()
    return nc


def kernel(**inputs):
    if "nc" not in _cache:
        _cache["nc"] = _build()
    nc = _cache["nc"]
    from concourse.bass_utils import run_bass_kernel_spmd

    names = ["Wq", "bq", "Wk", "bk", "Wv", "bv", "Wo", "bo", "gamma", "beta"]
    shared = {n: np.ascontiguousarray(np.asarray(inputs[n], dtype=np.float32))
              for n in names}
    x = np.asarray(inputs["x"], dtype=np.float32)
    in_maps = [dict(shared, x=np.ascontiguousarray(x[b])) for b in range(N_CORES)]
    res = run_bass_kernel_spmd(nc, in_maps, core_ids=list(range(N_CORES)))
    out = np.stack([res.results[i]["out"] for i in range(N_CORES)], axis=0)
    return out.astype(np.float32)



# revision 34
# speedup vs baseline: 1.1280x; 1.0231x over previous
"""Local multi-headed attention (window +/-2) + residual + LayerNorm, Trainium2 Bass kernel.

Sharding: data-parallel over batch. B=8 batch elements -> one per NeuronCore (8 cores).
Each core computes the full sequence for its batch element; no collectives.

Per-core layout strategy:
  - x is loaded naturally [s,d] and PE-transposed to xT [d,s] (bf16) for the projections.
  - Q/K/V projections: W.T @ xT -> qT/kT/vT in [d_out, s] layout (heads on partitions,
    2 heads of 64 dims per 128-partition tile), bf16 weights, fp32 PSUM accumulate,
    PSUM->SBUF copy on ScalarE fused with the per-partition bias add (casting to bf16).
  - Scores: per window offset w, elementwise prod = qT * shift_w(kT) (DVE bf16), then one
    matmul with a block-ones [128,128] matrix which simultaneously reduces over the 64
    head dims AND broadcasts the per-head score back to all 64 partitions.
  - Zero-padded sequence ends reproduce the reference's pad semantics exactly:
    k_pad = 0 @ Wk + bk, arranged by filling halo columns with the bias.
  - Softmax over the 5 offsets: exp on ScalarE (scale=1/8 fused), denominator summed
    over taps on TensorE via a (blockones/64) stationary accumulating in PSUM,
    reciprocal_approx_fast on DVE, AV = sum_w exp_w * shift_w(vT) on DVE.
  - O-projection: attT blocks as stationary operand against Wo ->
    o in [s,d] layout; bias bo folded into the residual x on GpSimd.
  - Residual + LayerNorm in [s,d] layout with free-dim reductions; gamma/beta applied
    from partition-broadcast copies (built once via a K=1 matmul).
"""
import os
import sys
import numpy as np

B, S, D = 8, 4096, 768
HEADS = 12
DH = 64
W = 5          # window taps, offsets -2..2
CHUNK = 256    # sequence chunk per inner iteration
NCH = S // CHUNK
DT = D // 128  # 6 partition tiles of d
EPS = 1e-5
N_CORES = 8

_cache = {}


def _build():
    import bass_rust
    import concourse.bass as bass
    import concourse.tile as tile
    from concourse import bacc, mybir
    from concourse.masks import make_identity

    def taps_ap(base, n_taps):
        """Overlapping-taps view: base [128, S] -> [128, n_taps, S-...]
        rows at element offsets 0, 2, 4, ... (stride 2), each CHUNK long."""
        pstride = base.ap[0][0]
        return bass_rust.AP(base.tensor, base.offset,
                            [[pstride, 128], [2, n_taps], [1, CHUNK]])

    f32 = mybir.dt.float32
    f32r = mybir.dt.float32r
    bf16 = mybir.dt.bfloat16
    AF = mybir.ActivationFunctionType
    ALU = mybir.AluOpType

    nc = bacc.Bacc("TRN2", target_bir_lowering=False, debug=False,
                   num_devices=N_CORES)

    x_ap = nc.dram_tensor("x", [S, D], f32, kind="ExternalInput").ap()
    wq_ap = nc.dram_tensor("Wq", [D, D], f32, kind="ExternalInput").ap()
    bq_ap = nc.dram_tensor("bq", [D], f32, kind="ExternalInput").ap()
    wk_ap = nc.dram_tensor("Wk", [D, D], f32, kind="ExternalInput").ap()
    bk_ap = nc.dram_tensor("bk", [D], f32, kind="ExternalInput").ap()
    wv_ap = nc.dram_tensor("Wv", [D, D], f32, kind="ExternalInput").ap()
    bv_ap = nc.dram_tensor("bv", [D], f32, kind="ExternalInput").ap()
    wo_ap = nc.dram_tensor("Wo", [D, D], f32, kind="ExternalInput").ap()
    bo_ap = nc.dram_tensor("bo", [D], f32, kind="ExternalInput").ap()
    gamma_ap = nc.dram_tensor("gamma", [D], f32, kind="ExternalInput").ap()
    beta_ap = nc.dram_tensor("beta", [D], f32, kind="ExternalInput").ap()
    out_ap = nc.dram_tensor("out", [S, D], f32, kind="ExternalOutput").ap()

    with tile.TileContext(nc) as tc:
        # ---------------- persistent tiles ----------------
        with tc.tile_pool(name="persist", bufs=1) as pp:
            # weights: q/k/v in bf16 (score path tolerance), Wo in f32 (o path)
            wq_sb = pp.tile([128, DT, D], bf16, tag="wq")
            wk_sb = pp.tile([128, DT, D], bf16, tag="wk")
            wv_sb = pp.tile([128, DT, D], bf16, tag="wv")
            wo_sb = pp.tile([128, DT, D], bf16, tag="wo")
            bqT = pp.tile([128, DT], f32, tag="bqT")
            bkT = pp.tile([128, DT], f32, tag="bkT")
            bvT = pp.tile([128, DT], f32, tag="bvT")
            bo_sb = pp.tile([1, D], f32, tag="bo")
            g_sb = pp.tile([1, D], f32, tag="g")
            be_sb = pp.tile([1, D], f32, tag="be")
            ones_row = pp.tile([1, 128], f32, tag="ones")
            ones_bf = pp.tile([1, 128], bf16, tag="onesbf")
            bo_bf = pp.tile([1, D], bf16, tag="bobf")
            blockones = pp.tile([128, 128], bf16, tag="bones")
            blockones64 = pp.tile([128, 128], bf16, tag="bones64")
            gb_bc = pp.tile([128, D], f32, tag="gbbc")
            be_bc = pp.tile([128, D], f32, tag="bebc")

            for w_ap, sb in ((wq_ap, wq_sb), (wk_ap, wk_sb), (wv_ap, wv_sb),
                             (wo_ap, wo_sb)):
                # gpsimd DMA casts f32->bf16 in flight (no staging tile)
                nc.gpsimd.dma_start(sb[:], w_ap.rearrange("(kt p) n -> p kt n",
                                                          p=128))
            nc.sync.dma_start(bqT[:], bq_ap.rearrange("(t p) -> p t", p=128))
            nc.sync.dma_start(bkT[:], bk_ap.rearrange("(t p) -> p t", p=128))
            nc.sync.dma_start(bvT[:], bv_ap.rearrange("(t p) -> p t", p=128))
            nc.sync.dma_start(bo_sb[:], bo_ap[:])
            nc.sync.dma_start(g_sb[:], gamma_ap[:])
            nc.sync.dma_start(be_sb[:], beta_ap[:])

            nc.vector.memset(ones_row[:], 1.0)
            nc.vector.memset(ones_bf[:], 1.0)
            nc.vector.memset(blockones[:], 0.0)
            nc.vector.memset(blockones[0:64, 0:64], 1.0)
            nc.vector.memset(blockones[64:128, 64:128], 1.0)
            nc.vector.memset(blockones64[:], 0.0)
            nc.vector.memset(blockones64[0:64, 0:64], 1.0 / 64.0)
            nc.vector.memset(blockones64[64:128, 64:128], 1.0 / 64.0)

            # fold bv into bo: att = sum_w p_w (v0_w + bv) = sum_w p_w v0_w + bv
            # (softmax weights sum to 1), so (att+bv)@Wo = att@Wo + bv@Wo.
            # bo_total = bo + bv @ Wo, then broadcast across partitions.
            bvT_bf = pp.tile([128, DT], bf16, tag="bvbf")
            nc.vector.tensor_copy(bvT_bf[:], bvT[:])
            with tc.tile_pool(name="initps", bufs=1, space="PSUM") as initps:
                bvwo = initps.tile([1, D], f32, tag="bvwo")
                for kt in range(DT):
                    nc.tensor.matmul(bvwo[:, 0:512], bvT_bf[:, kt:kt + 1],
                                     wo_sb[:, kt, 0:512],
                                     start=(kt == 0), stop=(kt == DT - 1))
                for kt in range(DT):
                    nc.tensor.matmul(bvwo[:, 512:D], bvT_bf[:, kt:kt + 1],
                                     wo_sb[:, kt, 512:D],
                                     start=(kt == 0), stop=(kt == DT - 1))
                nc.vector.tensor_tensor(bo_sb[:], bo_sb[:], bvwo[:], ALU.add)
                nc.vector.tensor_copy(bo_bf[:], bo_sb[:])
                for src, dst in ((g_sb, gb_bc), (be_sb, be_bc)):
                    t = initps.tile([128, D], f32, tag="gbps")
                    nc.tensor.matmul(t[:, 0:512], ones_row[:], src[:, 0:512])
                    nc.tensor.matmul(t[:, 512:D], ones_row[:], src[:, 512:D])
                    nc.vector.tensor_copy(dst[:], t[:])

            # ---------------- working pools ----------------
            with tc.tile_pool(name="ppsum", bufs=2, space="PSUM") as ppsum, \
                 tc.tile_pool(name="spsum", bufs=1, space="PSUM") as spsum, \
                 tc.tile_pool(name="dnpsum", bufs=1, space="PSUM") as dnpsum, \
                 tc.tile_pool(name="opsum", bufs=1, space="PSUM") as opsum, \
                 tc.tile_pool(name="xpool", bufs=4) as xpool, \
                 tc.tile_pool(name="xbfpool", bufs=3) as xbfpool, \
                 tc.tile_pool(name="xtpool", bufs=3) as xtpool, \
                 tc.tile_pool(name="qpool", bufs=3) as qpool, \
                 tc.tile_pool(name="kvpool", bufs=4) as kvpool, \
                 tc.tile_pool(name="kvshpool", bufs=4) as kvshpool, \
                 tc.tile_pool(name="atpool", bufs=2) as atpool, \
                 tc.tile_pool(name="appool", bufs=2) as appool, \
                 tc.tile_pool(name="dnpool", bufs=3) as dnpool, \
                 tc.tile_pool(name="ypool", bufs=2) as ypool, \
                 tc.tile_pool(name="stpool", bufs=3) as stpool:

                kc_tiles = [None] * NCH
                vc_tiles = [None] * NCH
                ksh_tiles = [None] * NCH
                vsh_tiles = [None] * NCH

                def project(c):
                    """projections for chunk c -> qT (bf16) and kc/vc center cols."""
                    s0 = c * CHUNK
                    # load x naturally, 2 s-subtiles of 128 (scalar hwdge
                    # queue: keeps loads off the store-congested sync queue)
                    x_sb = xpool.tile([128, 2, D], f32, tag="x")
                    nc.scalar.dma_start(
                        x_sb[:], x_ap[s0:s0 + CHUNK, :].rearrange(
                            "(st p) d -> p st d", p=128))
                    # bf16 copy of x via gpsimd cast-DMA, then XBAR DMA
                    # transpose straight to xT (replaces PE transposes +
                    # ScalarE PSUM drains)
                    xbf = xbfpool.tile([128, 2, D], bf16, tag="xbf")
                    nc.gpsimd.dma_start(
                        xbf[:], x_ap[s0:s0 + CHUNK, :].rearrange(
                            "(st p) d -> p st d", p=128))
                    xT = xtpool.tile([128, DT, CHUNK], bf16, tag="xT")
                    for st in range(2):
                        nc.scalar.dma_start_transpose(
                            xT[:, :, st * 128:(st + 1) * 128], xbf[:, st, :])

                    # projections. K/V are BIAS-FREE: the k-bias shifts all 5
                    # scores of a token equally (softmax-invariant, pad slots
                    # score 0 = the shifted pad score), and the v-bias was
                    # folded into bo via bv@Wo since softmax weights sum to 1.
                    qT = qpool.tile([128, DT, CHUNK], bf16, tag="qT")
                    kc = kvpool.tile([128, DT, CHUNK + 4], bf16, tag="kc")
                    vc = kvpool.tile([128, DT, CHUNK + 4], bf16, tag="vc")
                    ksh = kvshpool.tile([128, DT, CHUNK + 2], bf16, tag="ksh")
                    vsh = kvshpool.tile([128, DT, CHUNK + 2], bf16, tag="vsh")
                    kc_tiles[c] = kc
                    vc_tiles[c] = vc
                    ksh_tiles[c] = ksh
                    vsh_tiles[c] = vsh
                    for (wsb, bT, dst) in ((wq_sb, bqT, qT),
                                           (wk_sb, None, kc),
                                           (wv_sb, None, vc)):
                        for dtp in range(DT // 2):
                            ps = ppsum.tile([128, 2, CHUNK], f32, tag="proj")
                            for pl in range(2):
                                dt = 2 * dtp + pl
                                for kt in range(DT):
                                    nc.tensor.matmul(
                                        ps[:, pl, :],
                                        wsb[:, kt, dt * 128:(dt + 1) * 128],
                                        xT[:, kt, :],
                                        start=(kt == 0), stop=(kt == DT - 1))
                            if bT is not None:
                                for pl in range(2):
                                    dt = 2 * dtp + pl
                                    nc.scalar.activation(
                                        dst[:, dt, :], ps[:, pl, :],
                                        AF.Identity, bias=bT[:, dt:dt + 1])
                            else:
                                nc.scalar.copy(
                                    dst[:, 2 * dtp:2 * dtp + 2, 2:2 + CHUNK],
                                    ps[:])
                    # halo fills (bias-free k/v values are chunk-consistent)
                    if c > 0:
                        # left halo of c <- tail of c-1 center; and
                        # right halo of c-1 <- head of c center
                        for big_prev, big_cur in ((kc_tiles[c - 1], kc),
                                                  (vc_tiles[c - 1], vc)):
                            nc.gpsimd.tensor_copy(big_cur[:, :, 0:2],
                                                  big_prev[:, :, CHUNK:CHUNK + 2])
                            nc.gpsimd.tensor_copy(big_prev[:, :, CHUNK + 2:CHUNK + 4],
                                                  big_cur[:, :, 2:4])
                        # c-1 tiles are now final: build its odd-tap shifted
                        # copies (element +1) so odd taps hit 2x DVE mode
                        nc.scalar.dma_start(ksh_tiles[c - 1][:],
                                            kc_tiles[c - 1][:, :, 1:3 + CHUNK])
                        nc.scalar.dma_start(vsh_tiles[c - 1][:],
                                            vc_tiles[c - 1][:, :, 1:3 + CHUNK])
                    if c == 0:
                        for big in (kc, vc):
                            nc.vector.memset(big[:, :, 0:2], 0.0)
                    if c == NCH - 1:
                        for big in (kc, vc):
                            nc.vector.memset(big[:, :, CHUNK + 2:CHUNK + 4], 0.0)
                        nc.scalar.dma_start(ksh[:], kc[:, :, 1:3 + CHUNK])
                        nc.scalar.dma_start(vsh[:], vc[:, :, 1:3 + CHUNK])
                    return x_sb, qT

                def attention(c, x_sb, qT):
                    """scores/softmax/AV/O-proj/LN for chunk c (projections done)."""
                    s0 = c * CHUNK
                    kc, vc = kc_tiles[c], vc_tiles[c]
                    ksh, vsh = ksh_tiles[c], vsh_tiles[c]
                    att = atpool.tile([128, DT, CHUNK], bf16, tag="att")
                    for dt in range(DT):
                        # products: merged even taps from kc, merged odd taps
                        # from the +1-shifted copy (both 2x bf16 aligned)
                        prod = appool.tile([128, W, CHUNK], bf16, tag="prod")
                        q_bc3 = qT[:, dt, :].unsqueeze(1).broadcast_to(
                            [128, 3, CHUNK])
                        q_bc2 = qT[:, dt, :].unsqueeze(1).broadcast_to(
                            [128, 2, CHUNK])
                        nc.vector.tensor_tensor(
                            prod[:, 0:W:2, :], q_bc3,
                            taps_ap(kc[:, dt, 0:CHUNK], 3), ALU.mult)
                        nc.vector.tensor_tensor(
                            prod[:, 1:W:2, :], q_bc2,
                            taps_ap(ksh[:, dt, 0:CHUNK], 2), ALU.mult)
                        # scores + head-reduce + broadcast: tap pairs share a
                        # matmul (512 f32 = exactly one PSUM bank each)
                        sc = spsum.tile([128, W, CHUNK], f32, tag="scores")
                        nc.tensor.matmul(sc[:, 0:2, :], blockones[:],
                                         prod[:, 0:2, :])
                        nc.tensor.matmul(sc[:, 2:4, :], blockones[:],
                                         prod[:, 2:4, :])
                        nc.tensor.matmul(sc[:, 4, :], blockones[:],
                                         prod[:, 4, :])
                        # exp with fused 1/sqrt(dh) scale, split in two so the
                        # first pair drains while taps 2-4 still matmul
                        ex = appool.tile([128, W, CHUNK], bf16, tag="exp")
                        nc.scalar.activation(ex[:, 0:2, :], sc[:, 0:2, :],
                                             AF.Exp, scale=0.125)
                        nc.scalar.activation(ex[:, 2:W, :], sc[:, 2:W, :],
                                             AF.Exp, scale=0.125)
                        # denominator: sum the 5 taps on TensorE via the
                        # (blockones/64) stationary (rows within a head block
                        # are identical, so the 64-row mean reproduces each
                        # tap exactly while PSUM accumulates over taps).
                        dn_ps = dnpsum.tile([128, CHUNK], f32, tag="dnps")
                        for w in range(W):
                            nc.tensor.matmul(dn_ps[:], blockones64[:],
                                             ex[:, w, :],
                                             start=(w == 0), stop=(w == W - 1))
                        rinv = dnpool.tile([128, CHUNK], f32, tag="rinv")
                        nc.vector.reciprocal_approx_fast(rinv[:], dn_ps[:])
                        # AV: avp_w = exp_w * v_tap_w (merged even/odd), then
                        # pairwise tap-sum tree
                        avp = appool.tile([128, W, CHUNK], bf16, tag="avp")
                        nc.vector.tensor_tensor(
                            avp[:, 0:W:2, :], ex[:, 0:W:2, :],
                            taps_ap(vc[:, dt, 0:CHUNK], 3), ALU.mult)
                        nc.vector.tensor_tensor(
                            avp[:, 1:W:2, :], ex[:, 1:W:2, :],
                            taps_ap(vsh[:, dt, 0:CHUNK], 2), ALU.mult)
                        pair = dnpool.tile([128, 2, CHUNK], bf16, tag="pair")
                        nc.vector.tensor_tensor(pair[:], avp[:, 0:2, :],
                                                avp[:, 2:4, :], ALU.add)
                        asum = dnpool.tile([128, CHUNK], bf16, tag="asum")
                        nc.vector.tensor_tensor(asum[:], pair[:, 0, :],
                                                pair[:, 1, :], ALU.add)
                        nc.vector.tensor_tensor(asum[:], asum[:], avp[:, 4, :],
                                                ALU.add)
                        nc.vector.tensor_tensor(att[:, dt, :], asum[:], rinv[:],
                                                ALU.mult)

                    # O-projection + bias + residual + LayerNorm per s-tile
                    for st in range(2):
                        op = opsum.tile([128, D], f32, tag="o")
                        for dt in range(DT):
                            a_blk = att[:, dt, st * 128:(st + 1) * 128]
                            nc.tensor.matmul(op[:, 0:512], a_blk,
                                             wo_sb[:, dt, 0:512],
                                             start=(dt == 0), stop=False)
                            nc.tensor.matmul(op[:, 512:D], a_blk,
                                             wo_sb[:, dt, 512:D],
                                             start=(dt == 0), stop=False)
                        nc.tensor.matmul(op[:, 0:512], ones_bf[:],
                                         bo_bf[:, 0:512], start=False, stop=True)
                        nc.tensor.matmul(op[:, 512:D], ones_bf[:],
                                         bo_bf[:, 512:D], start=False, stop=True)
                        ypre = ypool.tile([128, D], f32, tag="ypre")
                        nc.vector.tensor_tensor(ypre[:], op[:], x_sb[:, st, :],
                                                ALU.add)
                        # LayerNorm stats
                        stats = stpool.tile([128, 8], f32, tag="stats")
                        dump = stpool.tile([128, D], bf16, tag="dump")
                        nc.vector.tensor_reduce(stats[:, 0:1], ypre[:],
                                                axis=mybir.AxisListType.X,
                                                op=ALU.add)
                        nc.scalar.activation(dump[:], ypre[:], AF.Square,
                                             accum_out=stats[:, 1:2])
                        # var = (sumsq - sum^2/768)/768 ; rstd = 1/sqrt(var+eps)
                        nc.vector.tensor_tensor(stats[:, 2:3], stats[:, 0:1],
                                                stats[:, 0:1], ALU.mult)
                        nc.vector.tensor_scalar_mul(stats[:, 2:3], stats[:, 2:3],
                                                    -1.0 / D)
                        nc.vector.tensor_tensor(stats[:, 2:3], stats[:, 2:3],
                                                stats[:, 1:2], ALU.add)
                        nc.vector.tensor_scalar(stats[:, 3:4], stats[:, 2:3],
                                                1.0 / D, EPS, ALU.mult, ALU.add)
                        nc.scalar.sqrt(stats[:, 4:5], stats[:, 3:4])
                        nc.vector.reciprocal(stats[:, 5:6], stats[:, 4:5])
                        # negmurstd = -sum/D * rstd
                        nc.vector.tensor_tensor(stats[:, 6:7], stats[:, 0:1],
                                                stats[:, 5:6], ALU.mult)
                        nc.vector.tensor_scalar_mul(stats[:, 6:7], stats[:, 6:7],
                                                    -1.0 / D)
                        y1 = ypool.tile([128, D], f32, tag="y1")
                        nc.scalar.activation(y1[:], ypre[:], AF.Identity,
                                             bias=stats[:, 6:7],
                                             scale=stats[:, 5:6])
                        y2 = ypool.tile([128, D], f32, tag="y2")
                        nc.gpsimd.tensor_tensor(y2[:], y1[:], gb_bc[:], ALU.mult)
                        nc.gpsimd.tensor_tensor(y2[:], y2[:], be_bc[:], ALU.add)
                        nc.sync.dma_start(
                            out_ap[s0 + st * 128: s0 + (st + 1) * 128, :], y2[:])

                # run projections one chunk ahead of attention (right halo dep)
                pend = None
                for c in range(NCH):
                    cur = project(c)
                    if pend is not None:
                        attention(c - 1, *pend)
                    pend = cur
                attention(NCH - 1, *pend)

    nc.compile()
    return nc


def kernel(**inputs):
    if "nc" not in _cache:
        _cache["nc"] = _build()
    nc = _cache["nc"]
    from concourse.bass_utils import run_bass_kernel_spmd

    names = ["Wq", "bq", "Wk", "bk", "Wv", "bv", "Wo", "bo", "gamma", "beta"]
    shared = {n: np.ascontiguousarray(np.asarray(inputs[n], dtype=np.float32))
              for n in names}
    x = np.asarray(inputs["x"], dtype=np.float32)
    in_maps = [dict(shared, x=np.ascontiguousarray(x[b])) for b in range(N_CORES)]
    res = run_bass_kernel_spmd(nc, in_maps, core_ids=list(range(N_CORES)))
    out = np.stack([res.results[i]["out"] for i in range(N_CORES)], axis=0)
    return out.astype(np.float32)



# revision 41
# speedup vs baseline: 1.1753x; 1.0419x over previous
"""Local multi-headed attention (window +/-2) + residual + LayerNorm, Trainium2 Bass kernel.

Sharding: data-parallel over batch. B=8 batch elements -> one per NeuronCore (8 cores).
Each core computes the full sequence for its batch element; no collectives.

Per-core layout strategy:
  - x is loaded naturally [s,d] and PE-transposed to xT [d,s] (bf16) for the projections.
  - Q/K/V projections: W.T @ xT -> qT/kT/vT in [d_out, s] layout (heads on partitions,
    2 heads of 64 dims per 128-partition tile), bf16 weights, fp32 PSUM accumulate,
    PSUM->SBUF copy on ScalarE fused with the per-partition bias add (casting to bf16).
  - Scores: per window offset w, elementwise prod = qT * shift_w(kT) (DVE bf16), then one
    matmul with a block-ones [128,128] matrix which simultaneously reduces over the 64
    head dims AND broadcasts the per-head score back to all 64 partitions.
  - Zero-padded sequence ends reproduce the reference's pad semantics exactly:
    k_pad = 0 @ Wk + bk, arranged by filling halo columns with the bias.
  - Softmax over the 5 offsets: exp on ScalarE (scale=1/8 fused), denominator summed
    over taps on TensorE via a (blockones/64) stationary accumulating in PSUM,
    reciprocal_approx_fast on DVE, AV = sum_w exp_w * shift_w(vT) on DVE.
  - O-projection: attT blocks as stationary operand against Wo ->
    o in [s,d] layout; bias bo folded into the residual x on GpSimd.
  - Residual + LayerNorm in [s,d] layout with free-dim reductions; gamma/beta applied
    from partition-broadcast copies (built once via a K=1 matmul).
"""
import os
import sys
import numpy as np

B, S, D = 8, 4096, 768
HEADS = 12
DH = 64
W = 5          # window taps, offsets -2..2
CHUNK = 256    # sequence chunk per inner iteration
NCH = S // CHUNK
DT = D // 128  # 6 partition tiles of d
EPS = 1e-5
N_CORES = 8

_cache = {}


def _build(apply_gamma_beta=True):
    import bass_rust
    import concourse.bass as bass
    import concourse.tile as tile
    from concourse import bacc, mybir

    def taps_ap(base, n_taps):
        """Overlapping-taps view: base [128, S] -> [128, n_taps, S-...]
        rows at element offsets 0, 2, 4, ... (stride 2), each CHUNK long."""
        pstride = base.ap[0][0]
        return bass_rust.AP(base.tensor, base.offset,
                            [[pstride, 128], [2, n_taps], [1, CHUNK]])

    f32 = mybir.dt.float32
    f32r = mybir.dt.float32r
    bf16 = mybir.dt.bfloat16
    AF = mybir.ActivationFunctionType
    ALU = mybir.AluOpType

    nc = bacc.Bacc("TRN2", target_bir_lowering=False, debug=False,
                   num_devices=N_CORES)

    x_ap = nc.dram_tensor("x", [S, D], f32, kind="ExternalInput").ap()
    wq_ap = nc.dram_tensor("Wq", [D, D], f32, kind="ExternalInput").ap()
    bq_ap = nc.dram_tensor("bq", [D], f32, kind="ExternalInput").ap()
    wk_ap = nc.dram_tensor("Wk", [D, D], f32, kind="ExternalInput").ap()
    bk_ap = nc.dram_tensor("bk", [D], f32, kind="ExternalInput").ap()
    wv_ap = nc.dram_tensor("Wv", [D, D], f32, kind="ExternalInput").ap()
    bv_ap = nc.dram_tensor("bv", [D], f32, kind="ExternalInput").ap()
    wo_ap = nc.dram_tensor("Wo", [D, D], f32, kind="ExternalInput").ap()
    bo_ap = nc.dram_tensor("bo", [D], f32, kind="ExternalInput").ap()
    gamma_ap = nc.dram_tensor("gamma", [D], f32, kind="ExternalInput").ap()
    beta_ap = nc.dram_tensor("beta", [D], f32, kind="ExternalInput").ap()
    out_ap = nc.dram_tensor("out", [S, D], f32, kind="ExternalOutput").ap()

    with tile.TileContext(nc) as tc:
        # ---------------- persistent tiles ----------------
        with tc.tile_pool(name="persist", bufs=1) as pp:
            # weights: q/k/v in bf16 (score path tolerance), Wo in f32 (o path)
            wq_sb = pp.tile([128, DT, D], bf16, tag="wq")
            wk_sb = pp.tile([128, DT, D], bf16, tag="wk")
            wv_sb = pp.tile([128, DT, D], bf16, tag="wv")
            wo_sb = pp.tile([128, DT, D], bf16, tag="wo")
            bqT = pp.tile([128, DT], f32, tag="bqT")
            bkT = pp.tile([128, DT], f32, tag="bkT")
            bvT = pp.tile([128, DT], f32, tag="bvT")
            bo_sb = pp.tile([1, D], f32, tag="bo")
            g_sb = pp.tile([1, D], f32, tag="g")
            be_sb = pp.tile([1, D], f32, tag="be")
            ones_row = pp.tile([1, 128], f32, tag="ones")
            ones_bf = pp.tile([1, 128], bf16, tag="onesbf")
            bo_bf = pp.tile([1, D], bf16, tag="bobf")
            blockones = pp.tile([128, 128], bf16, tag="bones")
            blockones64 = pp.tile([128, 128], bf16, tag="bones64")
            gb_bc = pp.tile([128, D], f32, tag="gbbc")
            be_bc = pp.tile([128, D], f32, tag="bebc")

            for w_ap, sb in ((wq_ap, wq_sb), (wk_ap, wk_sb), (wv_ap, wv_sb),
                             (wo_ap, wo_sb)):
                # gpsimd DMA casts f32->bf16 in flight (no staging tile)
                nc.gpsimd.dma_start(sb[:], w_ap.rearrange("(kt p) n -> p kt n",
                                                          p=128))
            nc.sync.dma_start(bqT[:], bq_ap.rearrange("(t p) -> p t", p=128))
            nc.sync.dma_start(bkT[:], bk_ap.rearrange("(t p) -> p t", p=128))
            nc.sync.dma_start(bvT[:], bv_ap.rearrange("(t p) -> p t", p=128))
            nc.sync.dma_start(bo_sb[:], bo_ap[:])
            nc.sync.dma_start(g_sb[:], gamma_ap[:])
            nc.sync.dma_start(be_sb[:], beta_ap[:])

            nc.vector.memset(ones_row[:], 1.0)
            nc.vector.memset(ones_bf[:], 1.0)
            nc.vector.memset(blockones[:], 0.0)
            nc.vector.memset(blockones[0:64, 0:64], 1.0)
            nc.vector.memset(blockones[64:128, 64:128], 1.0)
            nc.vector.memset(blockones64[:], 0.0)
            nc.vector.memset(blockones64[0:64, 0:64], 1.0 / 64.0)
            nc.vector.memset(blockones64[64:128, 64:128], 1.0 / 64.0)

            # fold bv into bo: att = sum_w p_w (v0_w + bv) = sum_w p_w v0_w + bv
            # (softmax weights sum to 1), so (att+bv)@Wo = att@Wo + bv@Wo.
            # bo_total = bo + bv @ Wo, then broadcast across partitions.
            bvT_bf = pp.tile([128, DT], bf16, tag="bvbf")
            nc.vector.tensor_copy(bvT_bf[:], bvT[:])
            with tc.tile_pool(name="initps", bufs=1, space="PSUM") as initps:
                bvwo = initps.tile([1, D], f32, tag="bvwo")
                for kt in range(DT):
                    nc.tensor.matmul(bvwo[:, 0:512], bvT_bf[:, kt:kt + 1],
                                     wo_sb[:, kt, 0:512],
                                     start=(kt == 0), stop=(kt == DT - 1))
                for kt in range(DT):
                    nc.tensor.matmul(bvwo[:, 512:D], bvT_bf[:, kt:kt + 1],
                                     wo_sb[:, kt, 512:D],
                                     start=(kt == 0), stop=(kt == DT - 1))
                nc.vector.tensor_tensor(bo_sb[:], bo_sb[:], bvwo[:], ALU.add)
                nc.vector.tensor_copy(bo_bf[:], bo_sb[:])
                for src, dst in ((g_sb, gb_bc), (be_sb, be_bc)):
                    t = initps.tile([128, D], f32, tag="gbps")
                    nc.tensor.matmul(t[:, 0:512], ones_row[:], src[:, 0:512])
                    nc.tensor.matmul(t[:, 512:D], ones_row[:], src[:, 512:D])
                    nc.vector.tensor_copy(dst[:], t[:])

            # ---------------- working pools ----------------
            with tc.tile_pool(name="ppsum", bufs=2, space="PSUM") as ppsum, \
                 tc.tile_pool(name="spsum", bufs=1, space="PSUM") as spsum, \
                 tc.tile_pool(name="dnpsum", bufs=1, space="PSUM") as dnpsum, \
                 tc.tile_pool(name="opsum", bufs=1, space="PSUM") as opsum, \
                 tc.tile_pool(name="xpool", bufs=4) as xpool, \
                 tc.tile_pool(name="xbfpool", bufs=3) as xbfpool, \
                 tc.tile_pool(name="xtpool", bufs=3) as xtpool, \
                 tc.tile_pool(name="qpool", bufs=3) as qpool, \
                 tc.tile_pool(name="kvpool", bufs=4) as kvpool, \
                 tc.tile_pool(name="kvshpool", bufs=4) as kvshpool, \
                 tc.tile_pool(name="atpool", bufs=2) as atpool, \
                 tc.tile_pool(name="appool", bufs=2) as appool, \
                 tc.tile_pool(name="dnpool", bufs=3) as dnpool, \
                 tc.tile_pool(name="ypool", bufs=2) as ypool, \
                 tc.tile_pool(name="stpool", bufs=3) as stpool:

                kc_tiles = [None] * NCH
                vc_tiles = [None] * NCH
                ksh_tiles = [None] * NCH
                vsh_tiles = [None] * NCH

                def project(c):
                    """projections for chunk c -> qT (bf16) and kc/vc center cols."""
                    s0 = c * CHUNK
                    # load x naturally, 2 s-subtiles of 128 (scalar hwdge
                    # queue: keeps loads off the store-congested sync queue)
                    x_sb = xpool.tile([128, 2, D], f32, tag="x")
                    nc.sync.dma_start(
                        x_sb[:], x_ap[s0:s0 + CHUNK, :].rearrange(
                            "(st p) d -> p st d", p=128))
                    # bf16 copy of x via gpsimd cast-DMA, then XBAR DMA
                    # transpose straight to xT (replaces PE transposes +
                    # ScalarE PSUM drains)
                    xbf = xbfpool.tile([128, 2, D], bf16, tag="xbf")
                    nc.gpsimd.dma_start(
                        xbf[:], x_ap[s0:s0 + CHUNK, :].rearrange(
                            "(st p) d -> p st d", p=128))
                    xT = xtpool.tile([128, DT, CHUNK], bf16, tag="xT")
                    for st in range(2):
                        nc.sync.dma_start_transpose(
                            xT[:, :, st * 128:(st + 1) * 128], xbf[:, st, :])

                    # projections. K/V are BIAS-FREE: the k-bias shifts all 5
                    # scores of a token equally (softmax-invariant, pad slots
                    # score 0 = the shifted pad score), and the v-bias was
                    # folded into bo via bv@Wo since softmax weights sum to 1.
                    qT = qpool.tile([128, DT, CHUNK], bf16, tag="qT")
                    kc = kvpool.tile([128, DT, CHUNK + 4], bf16, tag="kc")
                    vc = kvpool.tile([128, DT, CHUNK + 4], bf16, tag="vc")
                    ksh = kvshpool.tile([128, DT, CHUNK + 2], bf16, tag="ksh")
                    vsh = kvshpool.tile([128, DT, CHUNK + 2], bf16, tag="vsh")
                    kc_tiles[c] = kc
                    vc_tiles[c] = vc
                    ksh_tiles[c] = ksh
                    vsh_tiles[c] = vsh
                    for (wsb, bT, dst) in ((wq_sb, bqT, qT),
                                           (wk_sb, None, kc),
                                           (wv_sb, None, vc)):
                        for dtp in range(DT // 2):
                            ps = ppsum.tile([128, 2, CHUNK], f32, tag="proj")
                            for pl in range(2):
                                dt = 2 * dtp + pl
                                for kt in range(DT):
                                    nc.tensor.matmul(
                                        ps[:, pl, :],
                                        wsb[:, kt, dt * 128:(dt + 1) * 128],
                                        xT[:, kt, :],
                                        start=(kt == 0), stop=(kt == DT - 1))
                            if bT is not None:
                                for pl in range(2):
                                    dt = 2 * dtp + pl
                                    nc.scalar.activation(
                                        dst[:, dt, :], ps[:, pl, :],
                                        AF.Identity, bias=bT[:, dt:dt + 1])
                            else:
                                nc.scalar.copy(
                                    dst[:, 2 * dtp:2 * dtp + 2, 2:2 + CHUNK],
                                    ps[:])
                    # halo fills (bias-free k/v values are chunk-consistent)
                    if c > 0:
                        # left halo of c <- tail of c-1 center; and
                        # right halo of c-1 <- head of c center
                        for big_prev, big_cur in ((kc_tiles[c - 1], kc),
                                                  (vc_tiles[c - 1], vc)):
                            nc.gpsimd.tensor_copy(big_cur[:, :, 0:2],
                                                  big_prev[:, :, CHUNK:CHUNK + 2])
                            nc.gpsimd.tensor_copy(big_prev[:, :, CHUNK + 2:CHUNK + 4],
                                                  big_cur[:, :, 2:4])
                        # c-1 tiles are now final: build its odd-tap shifted
                        # copies (element +1) so odd taps hit 2x DVE mode
                        nc.scalar.dma_start(ksh_tiles[c - 1][:],
                                            kc_tiles[c - 1][:, :, 1:3 + CHUNK])
                        nc.scalar.dma_start(vsh_tiles[c - 1][:],
                                            vc_tiles[c - 1][:, :, 1:3 + CHUNK])
                    if c == 0:
                        for big in (kc, vc):
                            nc.vector.memset(big[:, :, 0:2], 0.0)
                    if c == NCH - 1:
                        for big in (kc, vc):
                            nc.vector.memset(big[:, :, CHUNK + 2:CHUNK + 4], 0.0)
                        nc.scalar.dma_start(ksh[:], kc[:, :, 1:3 + CHUNK])
                        nc.scalar.dma_start(vsh[:], vc[:, :, 1:3 + CHUNK])
                    return x_sb, qT

                def attention(c, x_sb, qT):
                    """scores/softmax/AV/O-proj/LN for chunk c (projections done)."""
                    s0 = c * CHUNK
                    kc, vc = kc_tiles[c], vc_tiles[c]
                    ksh, vsh = ksh_tiles[c], vsh_tiles[c]
                    att = atpool.tile([128, DT, CHUNK], bf16, tag="att")
                    for dt in range(DT):
                        # products: merged even taps from kc, merged odd taps
                        # from the +1-shifted copy (both 2x bf16 aligned)
                        prod = appool.tile([128, W, CHUNK], bf16, tag="prod")
                        q_bc3 = qT[:, dt, :].unsqueeze(1).broadcast_to(
                            [128, 3, CHUNK])
                        q_bc2 = qT[:, dt, :].unsqueeze(1).broadcast_to(
                            [128, 2, CHUNK])
                        nc.vector.tensor_tensor(
                            prod[:, 0:W:2, :], q_bc3,
                            taps_ap(kc[:, dt, 0:CHUNK], 3), ALU.mult)
                        nc.vector.tensor_tensor(
                            prod[:, 1:W:2, :], q_bc2,
                            taps_ap(ksh[:, dt, 0:CHUNK], 2), ALU.mult)
                        # scores + head-reduce + broadcast: tap pairs share a
                        # matmul (512 f32 = exactly one PSUM bank each)
                        sc = spsum.tile([128, W, CHUNK], f32, tag="scores")
                        nc.tensor.matmul(sc[:, 0:2, :], blockones[:],
                                         prod[:, 0:2, :])
                        nc.tensor.matmul(sc[:, 2:4, :], blockones[:],
                                         prod[:, 2:4, :])
                        nc.tensor.matmul(sc[:, 4, :], blockones[:],
                                         prod[:, 4, :])
                        # exp with fused 1/sqrt(dh) scale, split in two so the
                        # first pair drains while taps 2-4 still matmul
                        ex = appool.tile([128, W, CHUNK], bf16, tag="exp")
                        nc.scalar.activation(ex[:, 0:2, :], sc[:, 0:2, :],
                                             AF.Exp, scale=0.125)
                        nc.scalar.activation(ex[:, 2:W, :], sc[:, 2:W, :],
                                             AF.Exp, scale=0.125)
                        # denominator: sum the 5 taps on TensorE via the
                        # (blockones/64) stationary (rows within a head block
                        # are identical, so the 64-row mean reproduces each
                        # tap exactly while PSUM accumulates over taps).
                        dn_ps = dnpsum.tile([128, CHUNK], f32, tag="dnps")
                        for w in range(W):
                            nc.tensor.matmul(dn_ps[:], blockones64[:],
                                             ex[:, w, :],
                                             start=(w == 0), stop=(w == W - 1))
                        rinv = dnpool.tile([128, CHUNK], f32, tag="rinv")
                        nc.vector.reciprocal_approx_fast(rinv[:], dn_ps[:])
                        # AV: avp_w = exp_w * v_tap_w (merged even/odd), then
                        # pairwise tap-sum tree
                        avp = appool.tile([128, W, CHUNK], bf16, tag="avp")
                        nc.vector.tensor_tensor(
                            avp[:, 0:W:2, :], ex[:, 0:W:2, :],
                            taps_ap(vc[:, dt, 0:CHUNK], 3), ALU.mult)
                        nc.vector.tensor_tensor(
                            avp[:, 1:W:2, :], ex[:, 1:W:2, :],
                            taps_ap(vsh[:, dt, 0:CHUNK], 2), ALU.mult)
                        pair = dnpool.tile([128, 2, CHUNK], bf16, tag="pair")
                        nc.vector.tensor_tensor(pair[:], avp[:, 0:2, :],
                                                avp[:, 2:4, :], ALU.add)
                        asum = dnpool.tile([128, CHUNK], bf16, tag="asum")
                        nc.vector.tensor_tensor(asum[:], pair[:, 0, :],
                                                pair[:, 1, :], ALU.add)
                        nc.vector.tensor_tensor(asum[:], asum[:], avp[:, 4, :],
                                                ALU.add)
                        nc.vector.tensor_tensor(att[:, dt, :], asum[:], rinv[:],
                                                ALU.mult)

                    # O-projection + bias + residual + LayerNorm per s-tile
                    for st in range(2):
                        op = opsum.tile([128, D], f32, tag="o")
                        for dt in range(DT):
                            a_blk = att[:, dt, st * 128:(st + 1) * 128]
                            nc.tensor.matmul(op[:, 0:512], a_blk,
                                             wo_sb[:, dt, 0:512],
                                             start=(dt == 0), stop=False)
                            nc.tensor.matmul(op[:, 512:D], a_blk,
                                             wo_sb[:, dt, 512:D],
                                             start=(dt == 0), stop=False)
                        nc.tensor.matmul(op[:, 0:512], ones_bf[:],
                                         bo_bf[:, 0:512], start=False, stop=True)
                        nc.tensor.matmul(op[:, 512:D], ones_bf[:],
                                         bo_bf[:, 512:D], start=False, stop=True)
                        ypre = ypool.tile([128, D], f32, tag="ypre")
                        nc.vector.tensor_tensor(ypre[:], op[:], x_sb[:, st, :],
                                                ALU.add)
                        # LayerNorm stats
                        stats = stpool.tile([128, 8], f32, tag="stats")
                        dump = stpool.tile([128, D], bf16, tag="dump")
                        nc.vector.tensor_reduce(stats[:, 0:1], ypre[:],
                                                axis=mybir.AxisListType.X,
                                                op=ALU.add)
                        nc.scalar.activation(dump[:], ypre[:], AF.Square,
                                             accum_out=stats[:, 1:2])
                        # var = (sumsq - sum^2/768)/768 ; rstd = 1/sqrt(var+eps)
                        nc.vector.tensor_tensor(stats[:, 2:3], stats[:, 0:1],
                                                stats[:, 0:1], ALU.mult)
                        nc.vector.tensor_scalar_mul(stats[:, 2:3], stats[:, 2:3],
                                                    -1.0 / D)
                        nc.vector.tensor_tensor(stats[:, 2:3], stats[:, 2:3],
                                                stats[:, 1:2], ALU.add)
                        nc.vector.tensor_scalar(stats[:, 3:4], stats[:, 2:3],
                                                1.0 / D, EPS, ALU.mult, ALU.add)
                        nc.scalar.sqrt(stats[:, 4:5], stats[:, 3:4])
                        nc.vector.reciprocal(stats[:, 5:6], stats[:, 4:5])
                        # negmurstd = -sum/D * rstd
                        nc.vector.tensor_tensor(stats[:, 6:7], stats[:, 0:1],
                                                stats[:, 5:6], ALU.mult)
                        nc.vector.tensor_scalar_mul(stats[:, 6:7], stats[:, 6:7],
                                                    -1.0 / D)
                        y1 = ypool.tile([128, D], f32, tag="y1")
                        nc.scalar.activation(y1[:], ypre[:], AF.Identity,
                                             bias=stats[:, 6:7],
                                             scale=stats[:, 5:6])
                        if apply_gamma_beta:
                            y2 = ypool.tile([128, D], f32, tag="y2")
                            nc.gpsimd.tensor_tensor(y2[:], y1[:], gb_bc[:],
                                                    ALU.mult)
                            nc.gpsimd.tensor_tensor(y2[:], y2[:], be_bc[:],
                                                    ALU.add)
                            out_tile = y2
                        else:
                            out_tile = y1
                        nc.sync.dma_start(
                            out_ap[s0 + st * 128: s0 + (st + 1) * 128, :],
                            out_tile[:])

                # run projections one chunk ahead of attention (right halo dep)
                pend = None
                for c in range(NCH):
                    cur = project(c)
                    if pend is not None:
                        attention(c - 1, *pend)
                    pend = cur
                attention(NCH - 1, *pend)

    nc.compile()
    return nc


def kernel(**inputs):
    # gamma==1 / beta==0 lets the final scale/shift be skipped exactly;
    # build the matching specialization for the actual input values.
    plain_gb = (np.allclose(np.asarray(inputs["gamma"]), 1.0) and
                np.allclose(np.asarray(inputs["beta"]), 0.0))
    key = "nc_plain" if plain_gb else "nc"
    if key not in _cache:
        _cache[key] = _build(apply_gamma_beta=not plain_gb)
    nc = _cache[key]
    from concourse.bass_utils import run_bass_kernel_spmd

    names = ["Wq", "bq", "Wk", "bk", "Wv", "bv", "Wo", "bo", "gamma", "beta"]
    shared = {n: np.ascontiguousarray(np.asarray(inputs[n], dtype=np.float32))
              for n in names}
    x = np.asarray(inputs["x"], dtype=np.float32)
    in_maps = [dict(shared, x=np.ascontiguousarray(x[b])) for b in range(N_CORES)]
    res = run_bass_kernel_spmd(nc, in_maps, core_ids=list(range(N_CORES)))
    out = np.stack([res.results[i]["out"] for i in range(N_CORES)], axis=0)
    return out.astype(np.float32)



# revision 43
# speedup vs baseline: 1.2265x; 1.0436x over previous
"""Local multi-headed attention (window +/-2) + residual + LayerNorm, Trainium2 Bass kernel.

Sharding: data-parallel over batch. B=8 batch elements -> one per NeuronCore (8 cores).
Each core computes the full sequence for its batch element; no collectives.

Per-core layout strategy:
  - x is loaded naturally [s,d] and PE-transposed to xT [d,s] (bf16) for the projections.
  - Q/K/V projections: W.T @ xT -> qT/kT/vT in [d_out, s] layout (heads on partitions,
    2 heads of 64 dims per 128-partition tile), bf16 weights, fp32 PSUM accumulate,
    PSUM->SBUF copy on ScalarE fused with the per-partition bias add (casting to bf16).
  - Scores: per window offset w, elementwise prod = qT * shift_w(kT) (DVE bf16), then one
    matmul with a block-ones [128,128] matrix which simultaneously reduces over the 64
    head dims AND broadcasts the per-head score back to all 64 partitions.
  - Zero-padded sequence ends reproduce the reference's pad semantics exactly:
    k_pad = 0 @ Wk + bk, arranged by filling halo columns with the bias.
  - Softmax over the 5 offsets: exp on ScalarE (scale=1/8 fused), denominator summed
    over taps on TensorE via a (blockones/64) stationary accumulating in PSUM,
    reciprocal_approx_fast on DVE, AV = sum_w exp_w * shift_w(vT) on DVE.
  - O-projection: attT blocks as stationary operand against Wo ->
    o in [s,d] layout; bias bo folded into the residual x on GpSimd.
  - Residual + LayerNorm in [s,d] layout with free-dim reductions; gamma/beta applied
    from partition-broadcast copies (built once via a K=1 matmul).
"""
import os
import sys
import numpy as np

B, S, D = 8, 4096, 768
HEADS = 12
DH = 64
W = 5          # window taps, offsets -2..2
CHUNK = 256    # sequence chunk per inner iteration
NCH = S // CHUNK
DT = D // 128  # 6 partition tiles of d
EPS = 1e-5
N_CORES = 8

_cache = {}


def _build(apply_gamma_beta=True):
    import bass_rust
    import concourse.bass as bass
    import concourse.tile as tile
    from concourse import bacc, mybir

    def taps_ap(base, n_taps):
        """Overlapping-taps view: base [128, S] -> [128, n_taps, S-...]
        rows at element offsets 0, 2, 4, ... (stride 2), each CHUNK long."""
        pstride = base.ap[0][0]
        return bass_rust.AP(base.tensor, base.offset,
                            [[pstride, 128], [2, n_taps], [1, CHUNK]])

    f32 = mybir.dt.float32
    f32r = mybir.dt.float32r
    bf16 = mybir.dt.bfloat16
    AF = mybir.ActivationFunctionType
    ALU = mybir.AluOpType

    nc = bacc.Bacc("TRN2", target_bir_lowering=False, debug=False,
                   num_devices=N_CORES)

    x_ap = nc.dram_tensor("x", [S, D], f32, kind="ExternalInput").ap()
    wq_ap = nc.dram_tensor("Wq", [D, D], f32, kind="ExternalInput").ap()
    bq_ap = nc.dram_tensor("bq", [D], f32, kind="ExternalInput").ap()
    wk_ap = nc.dram_tensor("Wk", [D, D], f32, kind="ExternalInput").ap()
    bk_ap = nc.dram_tensor("bk", [D], f32, kind="ExternalInput").ap()
    wv_ap = nc.dram_tensor("Wv", [D, D], f32, kind="ExternalInput").ap()
    bv_ap = nc.dram_tensor("bv", [D], f32, kind="ExternalInput").ap()
    wo_ap = nc.dram_tensor("Wo", [D, D], f32, kind="ExternalInput").ap()
    bo_ap = nc.dram_tensor("bo", [D], f32, kind="ExternalInput").ap()
    gamma_ap = nc.dram_tensor("gamma", [D], f32, kind="ExternalInput").ap()
    beta_ap = nc.dram_tensor("beta", [D], f32, kind="ExternalInput").ap()
    out_ap = nc.dram_tensor("out", [S, D], f32, kind="ExternalOutput").ap()

    with tile.TileContext(nc) as tc:
        # ---------------- persistent tiles ----------------
        with tc.tile_pool(name="persist", bufs=1) as pp:
            # weights: q/k/v in bf16 (score path tolerance), Wo in f32 (o path)
            wq_sb = pp.tile([128, DT, D], bf16, tag="wq")
            wk_sb = pp.tile([128, DT, D], bf16, tag="wk")
            wv_sb = pp.tile([128, DT, D], bf16, tag="wv")
            wo_sb = pp.tile([128, DT, D], bf16, tag="wo")
            bqT = pp.tile([128, DT], f32, tag="bqT")
            bkT = pp.tile([128, DT], f32, tag="bkT")
            bvT = pp.tile([128, DT], f32, tag="bvT")
            bo_sb = pp.tile([1, D], f32, tag="bo")
            g_sb = pp.tile([1, D], f32, tag="g")
            be_sb = pp.tile([1, D], f32, tag="be")
            ones_row = pp.tile([1, 128], f32, tag="ones")
            ones_bf = pp.tile([1, 128], bf16, tag="onesbf")
            bo_bf = pp.tile([1, D], bf16, tag="bobf")
            blockones = pp.tile([128, 128], bf16, tag="bones")
            blockones64 = pp.tile([128, 128], bf16, tag="bones64")
            gb_bc = pp.tile([128, D], f32, tag="gbbc")
            be_bc = pp.tile([128, D], f32, tag="bebc")

            for w_ap, sb in ((wq_ap, wq_sb), (wk_ap, wk_sb), (wv_ap, wv_sb),
                             (wo_ap, wo_sb)):
                # gpsimd DMA casts f32->bf16 in flight (no staging tile)
                nc.gpsimd.dma_start(sb[:], w_ap.rearrange("(kt p) n -> p kt n",
                                                          p=128))
            nc.sync.dma_start(bqT[:], bq_ap.rearrange("(t p) -> p t", p=128))
            nc.sync.dma_start(bkT[:], bk_ap.rearrange("(t p) -> p t", p=128))
            nc.sync.dma_start(bvT[:], bv_ap.rearrange("(t p) -> p t", p=128))
            nc.sync.dma_start(bo_sb[:], bo_ap[:])
            nc.sync.dma_start(g_sb[:], gamma_ap[:])
            nc.sync.dma_start(be_sb[:], beta_ap[:])

            nc.vector.memset(ones_row[:], 1.0)
            nc.vector.memset(ones_bf[:], 1.0)
            nc.vector.memset(blockones[:], 0.0)
            nc.vector.memset(blockones[0:64, 0:64], 1.0)
            nc.vector.memset(blockones[64:128, 64:128], 1.0)
            nc.vector.memset(blockones64[:], 0.0)
            nc.vector.memset(blockones64[0:64, 0:64], 1.0 / 64.0)
            nc.vector.memset(blockones64[64:128, 64:128], 1.0 / 64.0)

            # fold bv into bo: att = sum_w p_w (v0_w + bv) = sum_w p_w v0_w + bv
            # (softmax weights sum to 1), so (att+bv)@Wo = att@Wo + bv@Wo.
            # bo_total = bo + bv @ Wo, then broadcast across partitions.
            bvT_bf = pp.tile([128, DT], bf16, tag="bvbf")
            nc.vector.tensor_copy(bvT_bf[:], bvT[:])
            with tc.tile_pool(name="initps", bufs=1, space="PSUM") as initps:
                bvwo = initps.tile([1, D], f32, tag="bvwo")
                for kt in range(DT):
                    nc.tensor.matmul(bvwo[:, 0:512], bvT_bf[:, kt:kt + 1],
                                     wo_sb[:, kt, 0:512],
                                     start=(kt == 0), stop=(kt == DT - 1))
                for kt in range(DT):
                    nc.tensor.matmul(bvwo[:, 512:D], bvT_bf[:, kt:kt + 1],
                                     wo_sb[:, kt, 512:D],
                                     start=(kt == 0), stop=(kt == DT - 1))
                nc.vector.tensor_tensor(bo_sb[:], bo_sb[:], bvwo[:], ALU.add)
                nc.vector.tensor_copy(bo_bf[:], bo_sb[:])
                for src, dst in ((g_sb, gb_bc), (be_sb, be_bc)):
                    t = initps.tile([128, D], f32, tag="gbps")
                    nc.tensor.matmul(t[:, 0:512], ones_row[:], src[:, 0:512])
                    nc.tensor.matmul(t[:, 512:D], ones_row[:], src[:, 512:D])
                    nc.vector.tensor_copy(dst[:], t[:])

            # ---------------- working pools ----------------
            with tc.tile_pool(name="ppsum", bufs=2, space="PSUM") as ppsum, \
                 tc.tile_pool(name="spsum", bufs=1, space="PSUM") as spsum, \
                 tc.tile_pool(name="dnpsum", bufs=1, space="PSUM") as dnpsum, \
                 tc.tile_pool(name="opsum", bufs=1, space="PSUM") as opsum, \
                 tc.tile_pool(name="xpool", bufs=4) as xpool, \
                 tc.tile_pool(name="xbfpool", bufs=3) as xbfpool, \
                 tc.tile_pool(name="xtpool", bufs=3) as xtpool, \
                 tc.tile_pool(name="qpool", bufs=3) as qpool, \
                 tc.tile_pool(name="kvpool", bufs=4) as kvpool, \
                 tc.tile_pool(name="kvshpool", bufs=4) as kvshpool, \
                 tc.tile_pool(name="atpool", bufs=2) as atpool, \
                 tc.tile_pool(name="appool", bufs=2) as appool, \
                 tc.tile_pool(name="dnpool", bufs=3) as dnpool, \
                 tc.tile_pool(name="ypool", bufs=3) as ypool, \
                 tc.tile_pool(name="stpool", bufs=3) as stpool:

                kc_tiles = [None] * NCH
                vc_tiles = [None] * NCH
                ksh_tiles = [None] * NCH
                vsh_tiles = [None] * NCH

                def project(c):
                    """projections for chunk c -> qT (bf16) and kc/vc center cols."""
                    s0 = c * CHUNK
                    # load x naturally, 2 s-subtiles of 128 (scalar hwdge
                    # queue: keeps loads off the store-congested sync queue)
                    x_sb = xpool.tile([128, 2, D], f32, tag="x")
                    nc.sync.dma_start(
                        x_sb[:], x_ap[s0:s0 + CHUNK, :].rearrange(
                            "(st p) d -> p st d", p=128))
                    # bf16 copy of x via gpsimd cast-DMA, then XBAR DMA
                    # transpose straight to xT (replaces PE transposes +
                    # ScalarE PSUM drains)
                    xbf = xbfpool.tile([128, 2, D], bf16, tag="xbf")
                    nc.gpsimd.dma_start(
                        xbf[:], x_ap[s0:s0 + CHUNK, :].rearrange(
                            "(st p) d -> p st d", p=128))
                    xT = xtpool.tile([128, DT, CHUNK], bf16, tag="xT")
                    for st in range(2):
                        nc.sync.dma_start_transpose(
                            xT[:, :, st * 128:(st + 1) * 128], xbf[:, st, :])

                    # projections. K/V are BIAS-FREE: the k-bias shifts all 5
                    # scores of a token equally (softmax-invariant, pad slots
                    # score 0 = the shifted pad score), and the v-bias was
                    # folded into bo via bv@Wo since softmax weights sum to 1.
                    qT = qpool.tile([128, DT, CHUNK], bf16, tag="qT")
                    kc = kvpool.tile([128, DT, CHUNK + 4], bf16, tag="kc")
                    vc = kvpool.tile([128, DT, CHUNK + 4], bf16, tag="vc")
                    ksh = kvshpool.tile([128, DT, CHUNK + 2], bf16, tag="ksh")
                    vsh = kvshpool.tile([128, DT, CHUNK + 2], bf16, tag="vsh")
                    kc_tiles[c] = kc
                    vc_tiles[c] = vc
                    ksh_tiles[c] = ksh
                    vsh_tiles[c] = vsh
                    for (wsb, bT, dst) in ((wq_sb, bqT, qT),
                                           (wk_sb, None, kc),
                                           (wv_sb, None, vc)):
                        for dtp in range(DT // 2):
                            ps = ppsum.tile([128, 2, CHUNK], f32, tag="proj")
                            for pl in range(2):
                                dt = 2 * dtp + pl
                                for kt in range(DT):
                                    nc.tensor.matmul(
                                        ps[:, pl, :],
                                        wsb[:, kt, dt * 128:(dt + 1) * 128],
                                        xT[:, kt, :],
                                        start=(kt == 0), stop=(kt == DT - 1))
                            if bT is not None:
                                for pl in range(2):
                                    dt = 2 * dtp + pl
                                    nc.scalar.activation(
                                        dst[:, dt, :], ps[:, pl, :],
                                        AF.Identity, bias=bT[:, dt:dt + 1])
                            else:
                                nc.scalar.copy(
                                    dst[:, 2 * dtp:2 * dtp + 2, 2:2 + CHUNK],
                                    ps[:])
                    # halo fills (bias-free k/v values are chunk-consistent)
                    if c > 0:
                        # left halo of c <- tail of c-1 center; and
                        # right halo of c-1 <- head of c center
                        for big_prev, big_cur in ((kc_tiles[c - 1], kc),
                                                  (vc_tiles[c - 1], vc)):
                            nc.gpsimd.tensor_copy(big_cur[:, :, 0:2],
                                                  big_prev[:, :, CHUNK:CHUNK + 2])
                            nc.gpsimd.tensor_copy(big_prev[:, :, CHUNK + 2:CHUNK + 4],
                                                  big_cur[:, :, 2:4])
                        # c-1 tiles are now final: build its odd-tap shifted
                        # copies (element +1) so odd taps hit 2x DVE mode
                        nc.scalar.dma_start(ksh_tiles[c - 1][:],
                                            kc_tiles[c - 1][:, :, 1:3 + CHUNK])
                        nc.scalar.dma_start(vsh_tiles[c - 1][:],
                                            vc_tiles[c - 1][:, :, 1:3 + CHUNK])
                    if c == 0:
                        for big in (kc, vc):
                            nc.vector.memset(big[:, :, 0:2], 0.0)
                    if c == NCH - 1:
                        for big in (kc, vc):
                            nc.vector.memset(big[:, :, CHUNK + 2:CHUNK + 4], 0.0)
                        nc.scalar.dma_start(ksh[:], kc[:, :, 1:3 + CHUNK])
                        nc.scalar.dma_start(vsh[:], vc[:, :, 1:3 + CHUNK])
                    return x_sb, qT

                def attention(c, x_sb, qT):
                    """scores/softmax/AV/O-proj/LN for chunk c (projections done)."""
                    s0 = c * CHUNK
                    kc, vc = kc_tiles[c], vc_tiles[c]
                    ksh, vsh = ksh_tiles[c], vsh_tiles[c]
                    att = atpool.tile([128, DT, CHUNK], bf16, tag="att")
                    for dt in range(DT):
                        # products: merged even taps from kc, merged odd taps
                        # from the +1-shifted copy (both 2x bf16 aligned)
                        prod = appool.tile([128, W, CHUNK], bf16, tag="prod")
                        q_bc3 = qT[:, dt, :].unsqueeze(1).broadcast_to(
                            [128, 3, CHUNK])
                        q_bc2 = qT[:, dt, :].unsqueeze(1).broadcast_to(
                            [128, 2, CHUNK])
                        nc.vector.tensor_tensor(
                            prod[:, 0:W:2, :], q_bc3,
                            taps_ap(kc[:, dt, 0:CHUNK], 3), ALU.mult)
                        nc.vector.tensor_tensor(
                            prod[:, 1:W:2, :], q_bc2,
                            taps_ap(ksh[:, dt, 0:CHUNK], 2), ALU.mult)
                        # scores + head-reduce + broadcast: tap pairs share a
                        # matmul (512 f32 = exactly one PSUM bank each)
                        sc = spsum.tile([128, W, CHUNK], f32, tag="scores")
                        nc.tensor.matmul(sc[:, 0:2, :], blockones[:],
                                         prod[:, 0:2, :])
                        nc.tensor.matmul(sc[:, 2:4, :], blockones[:],
                                         prod[:, 2:4, :])
                        nc.tensor.matmul(sc[:, 4, :], blockones[:],
                                         prod[:, 4, :])
                        # exp with fused 1/sqrt(dh) scale, split in two so the
                        # first pair drains while taps 2-4 still matmul
                        ex = appool.tile([128, W, CHUNK], bf16, tag="exp")
                        nc.scalar.activation(ex[:, 0:2, :], sc[:, 0:2, :],
                                             AF.Exp, scale=0.125)
                        nc.scalar.activation(ex[:, 2:W, :], sc[:, 2:W, :],
                                             AF.Exp, scale=0.125)
                        # denominator: sum the 5 taps on TensorE via the
                        # (blockones/64) stationary (rows within a head block
                        # are identical, so the 64-row mean reproduces each
                        # tap exactly while PSUM accumulates over taps).
                        dn_ps = dnpsum.tile([128, CHUNK], f32, tag="dnps")
                        for w in range(W):
                            nc.tensor.matmul(dn_ps[:], blockones64[:],
                                             ex[:, w, :],
                                             start=(w == 0), stop=(w == W - 1))
                        rinv = dnpool.tile([128, CHUNK], f32, tag="rinv")
                        nc.vector.reciprocal_approx_fast(rinv[:], dn_ps[:])
                        # AV: avp_w = exp_w * v_tap_w (merged even/odd), then
                        # pairwise tap-sum tree
                        avp = appool.tile([128, W, CHUNK], bf16, tag="avp")
                        nc.vector.tensor_tensor(
                            avp[:, 0:W:2, :], ex[:, 0:W:2, :],
                            taps_ap(vc[:, dt, 0:CHUNK], 3), ALU.mult)
                        nc.vector.tensor_tensor(
                            avp[:, 1:W:2, :], ex[:, 1:W:2, :],
                            taps_ap(vsh[:, dt, 0:CHUNK], 2), ALU.mult)
                        pair = dnpool.tile([128, 2, CHUNK], bf16, tag="pair")
                        nc.vector.tensor_tensor(pair[:], avp[:, 0:2, :],
                                                avp[:, 2:4, :], ALU.add)
                        asum = dnpool.tile([128, CHUNK], bf16, tag="asum")
                        nc.vector.tensor_tensor(asum[:], pair[:, 0, :],
                                                pair[:, 1, :], ALU.add)
                        nc.vector.tensor_tensor(asum[:], asum[:], avp[:, 4, :],
                                                ALU.add)
                        nc.vector.tensor_tensor(att[:, dt, :], asum[:], rinv[:],
                                                ALU.mult)

                    # O-projection + bias + residual + LN stats per s-tile;
                    # the sqrt for BOTH s-tiles is batched into one ACT so the
                    # Exp<->Sqrt table-set swap happens once per chunk
                    stats = stpool.tile([128, 2, 8], f32, tag="stats")
                    ypres = []
                    for st in range(2):
                        op = opsum.tile([128, D], f32, tag="o")
                        for dt in range(DT):
                            a_blk = att[:, dt, st * 128:(st + 1) * 128]
                            nc.tensor.matmul(op[:, 0:512], a_blk,
                                             wo_sb[:, dt, 0:512],
                                             start=(dt == 0), stop=False)
                            nc.tensor.matmul(op[:, 512:D], a_blk,
                                             wo_sb[:, dt, 512:D],
                                             start=(dt == 0), stop=False)
                        nc.tensor.matmul(op[:, 0:512], ones_bf[:],
                                         bo_bf[:, 0:512], start=False, stop=True)
                        nc.tensor.matmul(op[:, 512:D], ones_bf[:],
                                         bo_bf[:, 512:D], start=False, stop=True)
                        ypre = ypool.tile([128, D], f32, tag="ypre")
                        ypres.append(ypre)
                        nc.vector.tensor_tensor(ypre[:], op[:], x_sb[:, st, :],
                                                ALU.add)
                        # LayerNorm stats
                        S_ = stats[:, st, :]
                        dump = stpool.tile([128, D], bf16, tag="dump")
                        nc.vector.tensor_reduce(S_[:, 0:1], ypre[:],
                                                axis=mybir.AxisListType.X,
                                                op=ALU.add)
                        nc.scalar.activation(dump[:], ypre[:], AF.Square,
                                             accum_out=S_[:, 1:2])
                        # var = (sumsq - sum^2/768)/768
                        nc.vector.tensor_tensor(S_[:, 2:3], S_[:, 0:1],
                                                S_[:, 0:1], ALU.mult)
                        nc.vector.tensor_scalar_mul(S_[:, 2:3], S_[:, 2:3],
                                                    -1.0 / D)
                        nc.vector.tensor_tensor(S_[:, 2:3], S_[:, 2:3],
                                                S_[:, 1:2], ALU.add)
                        nc.vector.tensor_scalar(S_[:, 3:4], S_[:, 2:3],
                                                1.0 / D, EPS, ALU.mult, ALU.add)
                    # rstd = 1/sqrt(var+eps), both s-tiles in one sqrt
                    nc.scalar.sqrt(stats[:, :, 4:5], stats[:, :, 3:4])
                    nc.vector.reciprocal(stats[:, :, 5:6], stats[:, :, 4:5])
                    for st in range(2):
                        S_ = stats[:, st, :]
                        ypre = ypres[st]
                        # negmurstd = -sum/D * rstd
                        nc.vector.tensor_tensor(S_[:, 6:7], S_[:, 0:1],
                                                S_[:, 5:6], ALU.mult)
                        nc.vector.tensor_scalar_mul(S_[:, 6:7], S_[:, 6:7],
                                                    -1.0 / D)
                        y1 = ypool.tile([128, D], f32, tag="y1")
                        nc.scalar.activation(y1[:], ypre[:], AF.Identity,
                                             bias=S_[:, 6:7],
                                             scale=S_[:, 5:6])
                        if apply_gamma_beta:
                            y2 = ypool.tile([128, D], f32, tag="y2")
                            nc.gpsimd.tensor_tensor(y2[:], y1[:], gb_bc[:],
                                                    ALU.mult)
                            nc.gpsimd.tensor_tensor(y2[:], y2[:], be_bc[:],
                                                    ALU.add)
                            out_tile = y2
                        else:
                            out_tile = y1
                        nc.sync.dma_start(
                            out_ap[s0 + st * 128: s0 + (st + 1) * 128, :],
                            out_tile[:])

                # run projections one chunk ahead of attention (right halo dep)
                pend = None
                for c in range(NCH):
                    cur = project(c)
                    if pend is not None:
                        attention(c - 1, *pend)
                    pend = cur
                attention(NCH - 1, *pend)

    nc.compile()
    return nc


def kernel(**inputs):
    # gamma==1 / beta==0 lets the final scale/shift be skipped exactly;
    # build the matching specialization for the actual input values.
    plain_gb = (np.allclose(np.asarray(inputs["gamma"]), 1.0) and
                np.allclose(np.asarray(inputs["beta"]), 0.0))
    key = "nc_plain" if plain_gb else "nc"
    if key not in _cache:
        _cache[key] = _build(apply_gamma_beta=not plain_gb)
    nc = _cache[key]
    from concourse.bass_utils import run_bass_kernel_spmd

    names = ["Wq", "bq", "Wk", "bk", "Wv", "bv", "Wo", "bo", "gamma", "beta"]
    shared = {n: np.ascontiguousarray(np.asarray(inputs[n], dtype=np.float32))
              for n in names}
    x = np.asarray(inputs["x"], dtype=np.float32)
    in_maps = [dict(shared, x=np.ascontiguousarray(x[b])) for b in range(N_CORES)]
    res = run_bass_kernel_spmd(nc, in_maps, core_ids=list(range(N_CORES)))
    out = np.stack([res.results[i]["out"] for i in range(N_CORES)], axis=0)
    return out.astype(np.float32)



# revision 49
# speedup vs baseline: 1.2287x; 1.0018x over previous
"""Local multi-headed attention (window +/-2) + residual + LayerNorm, Trainium2 Bass kernel.

Sharding: data-parallel over batch. B=8 batch elements -> one per NeuronCore (8 cores).
Each core computes the full sequence for its batch element; no collectives.

Per-core layout strategy:
  - x loads natural [s,d] (f32, residual) + a bf16 copy via gpsimd cast-DMA which is
    XBAR DMA-transposed straight to xT [d,s] (no PE transposes, no ScalarE drains).
  - Q/K/V projections: W-block stationary @ xT -> [d_out, s] layout (2 heads of 64 per
    128-partition tile), bf16, fp32 PSUM; K/V are BIAS-FREE: the k-bias shifts all 5
    scores of a token equally (softmax shift-invariance, pads score 0), and the v-bias
    commutes out of the tap-sum (softmax weights sum to 1) -> folded as bv@Wo into bo.
    K/V PSUM drains pair 2 dt-planes per ScalarE copy; Q keeps its per-dt biased drain.
  - Scores: merged-tap products via overlapping-stride APs (even taps from kc, odd taps
    from a +1-element-shifted DMA copy so both hit 2x bf16 DVE mode), then blockones
    matmuls (tap pairs share one 512-col matmul) reduce heads + broadcast scores.
  - Softmax: exp on ScalarE (scale=1/8, split 2+3 taps for PE/ScalarE pipelining),
    denominator = tap-sum on TensorE via a (blockones/64) stationary accumulating in
    PSUM, reciprocal_approx_fast on DVE; AV via merged-tap products + pairwise adds.
  - O-projection: att blocks stationary against Wo + ones-row bias matmul -> o [s,d].
  - Residual + LayerNorm in [s,d]; both s-tiles' rstd batched into ONE Sqrt per chunk
    (one Exp<->Sqrt ACT-table swap); gamma/beta applied on GpSimd, skipped entirely
    when the inputs are exactly gamma=1/beta=0 (runtime-dispatched specialization).
  - DMA queues split: loads/stores/XBAR on sync, shifted-copy builds on scalar hwdge.
"""
import os
import sys
import numpy as np

B, S, D = 8, 4096, 768
HEADS = 12
DH = 64
W = 5          # window taps, offsets -2..2
CHUNK = 256    # sequence chunk per inner iteration
NCH = S // CHUNK
DT = D // 128  # 6 partition tiles of d
EPS = 1e-5
N_CORES = 8

_cache = {}


def _build(apply_gamma_beta=True):
    import bass_rust
    import concourse.bass as bass
    import concourse.tile as tile
    from concourse import bacc, mybir

    def taps_ap(base, n_taps):
        """Overlapping-taps view: base [128, S] -> [128, n_taps, S-...]
        rows at element offsets 0, 2, 4, ... (stride 2), each CHUNK long."""
        pstride = base.ap[0][0]
        return bass_rust.AP(base.tensor, base.offset,
                            [[pstride, 128], [2, n_taps], [1, CHUNK]])

    f32 = mybir.dt.float32
    f32r = mybir.dt.float32r
    bf16 = mybir.dt.bfloat16
    AF = mybir.ActivationFunctionType
    ALU = mybir.AluOpType

    nc = bacc.Bacc("TRN2", target_bir_lowering=False, debug=False,
                   num_devices=N_CORES)

    x_ap = nc.dram_tensor("x", [S, D], f32, kind="ExternalInput").ap()
    wq_ap = nc.dram_tensor("Wq", [D, D], f32, kind="ExternalInput").ap()
    bq_ap = nc.dram_tensor("bq", [D], f32, kind="ExternalInput").ap()
    wk_ap = nc.dram_tensor("Wk", [D, D], f32, kind="ExternalInput").ap()
    bk_ap = nc.dram_tensor("bk", [D], f32, kind="ExternalInput").ap()
    wv_ap = nc.dram_tensor("Wv", [D, D], f32, kind="ExternalInput").ap()
    bv_ap = nc.dram_tensor("bv", [D], f32, kind="ExternalInput").ap()
    wo_ap = nc.dram_tensor("Wo", [D, D], f32, kind="ExternalInput").ap()
    bo_ap = nc.dram_tensor("bo", [D], f32, kind="ExternalInput").ap()
    gamma_ap = nc.dram_tensor("gamma", [D], f32, kind="ExternalInput").ap()
    beta_ap = nc.dram_tensor("beta", [D], f32, kind="ExternalInput").ap()
    out_ap = nc.dram_tensor("out", [S, D], f32, kind="ExternalOutput").ap()

    with tile.TileContext(nc) as tc:
        # ---------------- persistent tiles ----------------
        with tc.tile_pool(name="persist", bufs=1) as pp:
            # weights: q/k/v in bf16 (score path tolerance), Wo in f32 (o path)
            wq_sb = pp.tile([128, DT, D], bf16, tag="wq")
            wk_sb = pp.tile([128, DT, D], bf16, tag="wk")
            wv_sb = pp.tile([128, DT, D], bf16, tag="wv")
            wo_sb = pp.tile([128, DT, D], bf16, tag="wo")
            bqT = pp.tile([128, DT], f32, tag="bqT")
            bkT = pp.tile([128, DT], f32, tag="bkT")
            bvT = pp.tile([128, DT], f32, tag="bvT")
            bo_sb = pp.tile([1, D], f32, tag="bo")
            g_sb = pp.tile([1, D], f32, tag="g")
            be_sb = pp.tile([1, D], f32, tag="be")
            ones_row = pp.tile([1, 128], f32, tag="ones")
            ones_bf = pp.tile([1, 128], bf16, tag="onesbf")
            bo_bf = pp.tile([1, D], bf16, tag="bobf")
            blockones = pp.tile([128, 128], bf16, tag="bones")
            blockones64 = pp.tile([128, 128], bf16, tag="bones64")
            gb_bc = pp.tile([128, D], f32, tag="gbbc")
            be_bc = pp.tile([128, D], f32, tag="bebc")

            for w_ap, sb in ((wq_ap, wq_sb), (wk_ap, wk_sb), (wv_ap, wv_sb),
                             (wo_ap, wo_sb)):
                # gpsimd DMA casts f32->bf16 in flight (no staging tile)
                nc.gpsimd.dma_start(sb[:], w_ap.rearrange("(kt p) n -> p kt n",
                                                          p=128))
            nc.sync.dma_start(bqT[:], bq_ap.rearrange("(t p) -> p t", p=128))
            nc.sync.dma_start(bkT[:], bk_ap.rearrange("(t p) -> p t", p=128))
            nc.sync.dma_start(bvT[:], bv_ap.rearrange("(t p) -> p t", p=128))
            nc.sync.dma_start(bo_sb[:], bo_ap[:])
            nc.sync.dma_start(g_sb[:], gamma_ap[:])
            nc.sync.dma_start(be_sb[:], beta_ap[:])

            nc.vector.memset(ones_row[:], 1.0)
            nc.vector.memset(ones_bf[:], 1.0)
            nc.vector.memset(blockones[:], 0.0)
            nc.vector.memset(blockones[0:64, 0:64], 1.0)
            nc.vector.memset(blockones[64:128, 64:128], 1.0)
            nc.vector.memset(blockones64[:], 0.0)
            nc.vector.memset(blockones64[0:64, 0:64], 1.0 / 64.0)
            nc.vector.memset(blockones64[64:128, 64:128], 1.0 / 64.0)

            # fold bv into bo: att = sum_w p_w (v0_w + bv) = sum_w p_w v0_w + bv
            # (softmax weights sum to 1), so (att+bv)@Wo = att@Wo + bv@Wo.
            # bo_total = bo + bv @ Wo, then broadcast across partitions.
            bvT_bf = pp.tile([128, DT], bf16, tag="bvbf")
            nc.vector.tensor_copy(bvT_bf[:], bvT[:])
            with tc.tile_pool(name="initps", bufs=1, space="PSUM") as initps:
                bvwo = initps.tile([1, D], f32, tag="bvwo")
                for kt in range(DT):
                    nc.tensor.matmul(bvwo[:, 0:512], bvT_bf[:, kt:kt + 1],
                                     wo_sb[:, kt, 0:512],
                                     start=(kt == 0), stop=(kt == DT - 1))
                for kt in range(DT):
                    nc.tensor.matmul(bvwo[:, 512:D], bvT_bf[:, kt:kt + 1],
                                     wo_sb[:, kt, 512:D],
                                     start=(kt == 0), stop=(kt == DT - 1))
                nc.vector.tensor_tensor(bo_sb[:], bo_sb[:], bvwo[:], ALU.add)
                nc.vector.tensor_copy(bo_bf[:], bo_sb[:])
                for src, dst in ((g_sb, gb_bc), (be_sb, be_bc)):
                    t = initps.tile([128, D], f32, tag="gbps")
                    nc.tensor.matmul(t[:, 0:512], ones_row[:], src[:, 0:512])
                    nc.tensor.matmul(t[:, 512:D], ones_row[:], src[:, 512:D])
                    nc.vector.tensor_copy(dst[:], t[:])

            # ---------------- working pools ----------------
            with tc.tile_pool(name="ppsum", bufs=2, space="PSUM") as ppsum, \
                 tc.tile_pool(name="spsum", bufs=1, space="PSUM") as spsum, \
                 tc.tile_pool(name="dnpsum", bufs=1, space="PSUM") as dnpsum, \
                 tc.tile_pool(name="opsum", bufs=1, space="PSUM") as opsum, \
                 tc.tile_pool(name="xpool", bufs=4) as xpool, \
                 tc.tile_pool(name="xbfpool", bufs=3) as xbfpool, \
                 tc.tile_pool(name="xtpool", bufs=3) as xtpool, \
                 tc.tile_pool(name="qpool", bufs=3) as qpool, \
                 tc.tile_pool(name="kvpool", bufs=4) as kvpool, \
                 tc.tile_pool(name="kvshpool", bufs=4) as kvshpool, \
                 tc.tile_pool(name="atpool", bufs=2) as atpool, \
                 tc.tile_pool(name="appool", bufs=2) as appool, \
                 tc.tile_pool(name="dnpool", bufs=3) as dnpool, \
                 tc.tile_pool(name="ypool", bufs=3) as ypool, \
                 tc.tile_pool(name="stpool", bufs=2) as stpool:

                kc_tiles = [None] * NCH
                vc_tiles = [None] * NCH
                ksh_tiles = [None] * NCH
                vsh_tiles = [None] * NCH

                def project(c):
                    """projections for chunk c -> qT (bf16) and kc/vc center cols."""
                    s0 = c * CHUNK
                    # load x naturally, 2 s-subtiles of 128 (scalar hwdge
                    # queue: keeps loads off the store-congested sync queue)
                    x_sb = xpool.tile([128, 2, D], f32, tag="x")
                    nc.sync.dma_start(
                        x_sb[:], x_ap[s0:s0 + CHUNK, :].rearrange(
                            "(st p) d -> p st d", p=128))
                    # bf16 copy of x via gpsimd cast-DMA, then XBAR DMA
                    # transpose straight to xT (replaces PE transposes +
                    # ScalarE PSUM drains)
                    xbf = xbfpool.tile([128, 2, D], bf16, tag="xbf")
                    nc.gpsimd.dma_start(
                        xbf[:], x_ap[s0:s0 + CHUNK, :].rearrange(
                            "(st p) d -> p st d", p=128))
                    xT = xtpool.tile([128, DT, CHUNK], bf16, tag="xT")
                    for st in range(2):
                        nc.sync.dma_start_transpose(
                            xT[:, :, st * 128:(st + 1) * 128], xbf[:, st, :])

                    # projections. K/V are BIAS-FREE: the k-bias shifts all 5
                    # scores of a token equally (softmax-invariant, pad slots
                    # score 0 = the shifted pad score), and the v-bias was
                    # folded into bo via bv@Wo since softmax weights sum to 1.
                    qT = qpool.tile([128, DT, CHUNK], bf16, tag="qT")
                    kc = kvpool.tile([128, DT, CHUNK + 4], bf16, tag="kc")
                    vc = kvpool.tile([128, DT, CHUNK + 4], bf16, tag="vc")
                    ksh = kvshpool.tile([128, DT, CHUNK + 2], bf16, tag="ksh")
                    vsh = kvshpool.tile([128, DT, CHUNK + 2], bf16, tag="vsh")
                    kc_tiles[c] = kc
                    vc_tiles[c] = vc
                    ksh_tiles[c] = ksh
                    vsh_tiles[c] = vsh
                    for (wsb, bT, dst) in ((wq_sb, bqT, qT),
                                           (wk_sb, None, kc),
                                           (wv_sb, None, vc)):
                        for dtp in range(DT // 2):
                            ps = ppsum.tile([128, 2, CHUNK], f32, tag="proj")
                            for pl in range(2):
                                dt = 2 * dtp + pl
                                for kt in range(DT):
                                    nc.tensor.matmul(
                                        ps[:, pl, :],
                                        wsb[:, kt, dt * 128:(dt + 1) * 128],
                                        xT[:, kt, :],
                                        start=(kt == 0), stop=(kt == DT - 1))
                            if bT is not None:
                                for pl in range(2):
                                    dt = 2 * dtp + pl
                                    nc.scalar.activation(
                                        dst[:, dt, :], ps[:, pl, :],
                                        AF.Identity, bias=bT[:, dt:dt + 1])
                            else:
                                nc.scalar.copy(
                                    dst[:, 2 * dtp:2 * dtp + 2, 2:2 + CHUNK],
                                    ps[:])
                    # halo fills (bias-free k/v values are chunk-consistent)
                    if c > 0:
                        # left halo of c <- tail of c-1 center; and
                        # right halo of c-1 <- head of c center
                        for big_prev, big_cur in ((kc_tiles[c - 1], kc),
                                                  (vc_tiles[c - 1], vc)):
                            nc.gpsimd.tensor_copy(big_cur[:, :, 0:2],
                                                  big_prev[:, :, CHUNK:CHUNK + 2])
                            nc.gpsimd.tensor_copy(big_prev[:, :, CHUNK + 2:CHUNK + 4],
                                                  big_cur[:, :, 2:4])
                        # c-1 tiles are now final: build its odd-tap shifted
                        # copies (element +1) so odd taps hit 2x DVE mode
                        nc.scalar.dma_start(ksh_tiles[c - 1][:],
                                            kc_tiles[c - 1][:, :, 1:3 + CHUNK])
                        nc.scalar.dma_start(vsh_tiles[c - 1][:],
                                            vc_tiles[c - 1][:, :, 1:3 + CHUNK])
                    if c == 0:
                        for big in (kc, vc):
                            nc.vector.memset(big[:, :, 0:2], 0.0)
                    if c == NCH - 1:
                        for big in (kc, vc):
                            nc.vector.memset(big[:, :, CHUNK + 2:CHUNK + 4], 0.0)
                        nc.scalar.dma_start(ksh[:], kc[:, :, 1:3 + CHUNK])
                        nc.scalar.dma_start(vsh[:], vc[:, :, 1:3 + CHUNK])
                    return x_sb, qT

                def attention(c, x_sb, qT):
                    """scores/softmax/AV/O-proj/LN for chunk c (projections done)."""
                    s0 = c * CHUNK
                    kc, vc = kc_tiles[c], vc_tiles[c]
                    ksh, vsh = ksh_tiles[c], vsh_tiles[c]
                    att = atpool.tile([128, DT, CHUNK], bf16, tag="att")
                    for dt in range(DT):
                        # products: merged even taps from kc, merged odd taps
                        # from the +1-shifted copy (both 2x bf16 aligned)
                        prod = appool.tile([128, W, CHUNK], bf16, tag="prod")
                        q_bc3 = qT[:, dt, :].unsqueeze(1).broadcast_to(
                            [128, 3, CHUNK])
                        q_bc2 = qT[:, dt, :].unsqueeze(1).broadcast_to(
                            [128, 2, CHUNK])
                        nc.vector.tensor_tensor(
                            prod[:, 0:W:2, :], q_bc3,
                            taps_ap(kc[:, dt, 0:CHUNK], 3), ALU.mult)
                        nc.vector.tensor_tensor(
                            prod[:, 1:W:2, :], q_bc2,
                            taps_ap(ksh[:, dt, 0:CHUNK], 2), ALU.mult)
                        # scores + head-reduce + broadcast: tap pairs share a
                        # matmul (512 f32 = exactly one PSUM bank each)
                        sc = spsum.tile([128, W, CHUNK], f32, tag="scores")
                        nc.tensor.matmul(sc[:, 0:2, :], blockones[:],
                                         prod[:, 0:2, :])
                        nc.tensor.matmul(sc[:, 2:4, :], blockones[:],
                                         prod[:, 2:4, :])
                        nc.tensor.matmul(sc[:, 4, :], blockones[:],
                                         prod[:, 4, :])
                        # exp with fused 1/sqrt(dh) scale, split in two so the
                        # first pair drains while taps 2-4 still matmul
                        ex = appool.tile([128, W, CHUNK], bf16, tag="exp")
                        nc.scalar.activation(ex[:, 0:2, :], sc[:, 0:2, :],
                                             AF.Exp, scale=0.125)
                        nc.scalar.activation(ex[:, 2:W, :], sc[:, 2:W, :],
                                             AF.Exp, scale=0.125)
                        # denominator: sum the 5 taps on TensorE via the
                        # (blockones/64) stationary (rows within a head block
                        # are identical, so the 64-row mean reproduces each
                        # tap exactly while PSUM accumulates over taps).
                        dn_ps = dnpsum.tile([128, CHUNK], f32, tag="dnps")
                        for w in range(W):
                            nc.tensor.matmul(dn_ps[:], blockones64[:],
                                             ex[:, w, :],
                                             start=(w == 0), stop=(w == W - 1))
                        rinv = dnpool.tile([128, CHUNK], f32, tag="rinv")
                        nc.vector.reciprocal_approx_fast(rinv[:], dn_ps[:])
                        # AV: avp_w = exp_w * v_tap_w (merged even/odd), then
                        # pairwise tap-sum tree
                        avp = appool.tile([128, W, CHUNK], bf16, tag="avp")
                        nc.vector.tensor_tensor(
                            avp[:, 0:W:2, :], ex[:, 0:W:2, :],
                            taps_ap(vc[:, dt, 0:CHUNK], 3), ALU.mult)
                        nc.vector.tensor_tensor(
                            avp[:, 1:W:2, :], ex[:, 1:W:2, :],
                            taps_ap(vsh[:, dt, 0:CHUNK], 2), ALU.mult)
                        pair = dnpool.tile([128, 2, CHUNK], bf16, tag="pair")
                        nc.vector.tensor_tensor(pair[:], avp[:, 0:2, :],
                                                avp[:, 2:4, :], ALU.add)
                        asum = dnpool.tile([128, CHUNK], bf16, tag="asum")
                        nc.vector.tensor_tensor(asum[:], pair[:, 0, :],
                                                pair[:, 1, :], ALU.add)
                        nc.vector.tensor_tensor(asum[:], asum[:], avp[:, 4, :],
                                                ALU.add)
                        nc.vector.tensor_tensor(att[:, dt, :], asum[:], rinv[:],
                                                ALU.mult)

                    # O-projection + bias + residual + LN stats per s-tile;
                    # the sqrt for BOTH s-tiles is batched into one ACT so the
                    # Exp<->Sqrt table-set swap happens once per chunk
                    stats = stpool.tile([128, 2, 8], f32, tag="stats")
                    ypres = []
                    for st in range(2):
                        op = opsum.tile([128, D], f32, tag="o")
                        for dt in range(DT):
                            a_blk = att[:, dt, st * 128:(st + 1) * 128]
                            nc.tensor.matmul(op[:, 0:512], a_blk,
                                             wo_sb[:, dt, 0:512],
                                             start=(dt == 0), stop=False)
                            nc.tensor.matmul(op[:, 512:D], a_blk,
                                             wo_sb[:, dt, 512:D],
                                             start=(dt == 0), stop=False)
                        nc.tensor.matmul(op[:, 0:512], ones_bf[:],
                                         bo_bf[:, 0:512], start=False, stop=True)
                        nc.tensor.matmul(op[:, 512:D], ones_bf[:],
                                         bo_bf[:, 512:D], start=False, stop=True)
                        ypre = ypool.tile([128, D], f32, tag="ypre")
                        ypres.append(ypre)
                        nc.vector.tensor_tensor(ypre[:], op[:], x_sb[:, st, :],
                                                ALU.add)
                        # LayerNorm stats: sum via ScalarE Identity-accum
                        # (frees DVE), sumsq via ScalarE Square-accum
                        S_ = stats[:, st, :]
                        dump = stpool.tile([128, D], bf16, tag="dump")
                        nc.scalar.activation(dump[:], ypre[:], AF.Identity,
                                             accum_out=S_[:, 0:1])
                        dump2 = stpool.tile([128, D], bf16, tag="dump2")
                        nc.scalar.activation(dump2[:], ypre[:], AF.Square,
                                             accum_out=S_[:, 1:2])
                        # var = (sumsq - sum^2/768)/768
                        nc.vector.tensor_tensor(S_[:, 2:3], S_[:, 0:1],
                                                S_[:, 0:1], ALU.mult)
                        nc.vector.tensor_scalar_mul(S_[:, 2:3], S_[:, 2:3],
                                                    -1.0 / D)
                        nc.vector.tensor_tensor(S_[:, 2:3], S_[:, 2:3],
                                                S_[:, 1:2], ALU.add)
                        nc.vector.tensor_scalar(S_[:, 3:4], S_[:, 2:3],
                                                1.0 / D, EPS, ALU.mult, ALU.add)
                    # rstd = 1/sqrt(var+eps), both s-tiles in one sqrt
                    nc.scalar.sqrt(stats[:, :, 4:5], stats[:, :, 3:4])
                    nc.vector.reciprocal(stats[:, :, 5:6], stats[:, :, 4:5])
                    for st in range(2):
                        S_ = stats[:, st, :]
                        ypre = ypres[st]
                        # negmurstd = -sum/D * rstd
                        nc.vector.tensor_tensor(S_[:, 6:7], S_[:, 0:1],
                                                S_[:, 5:6], ALU.mult)
                        nc.vector.tensor_scalar_mul(S_[:, 6:7], S_[:, 6:7],
                                                    -1.0 / D)
                        # y1 = ypre*rstd + negmurstd on DVE (2-port f32 mode;
                        # cheaper than the ScalarE activation)
                        y1 = ypool.tile([128, D], f32, tag="y1")
                        nc.vector.tensor_scalar(y1[:], ypre[:],
                                                S_[:, 5:6], S_[:, 6:7],
                                                ALU.mult, ALU.add)
                        if apply_gamma_beta:
                            y2 = ypool.tile([128, D], f32, tag="y2")
                            nc.gpsimd.tensor_tensor(y2[:], y1[:], gb_bc[:],
                                                    ALU.mult)
                            nc.gpsimd.tensor_tensor(y2[:], y2[:], be_bc[:],
                                                    ALU.add)
                            out_tile = y2
                        else:
                            out_tile = y1
                        nc.sync.dma_start(
                            out_ap[s0 + st * 128: s0 + (st + 1) * 128, :],
                            out_tile[:])

                # run projections one chunk ahead of attention (right halo dep)
                pend = None
                for c in range(NCH):
                    cur = project(c)
                    if pend is not None:
                        attention(c - 1, *pend)
                    pend = cur
                attention(NCH - 1, *pend)

    nc.compile()
    return nc


def kernel(**inputs):
    # gamma==1 / beta==0 lets the final scale/shift be skipped exactly;
    # build the matching specialization for the actual input values.
    plain_gb = (np.allclose(np.asarray(inputs["gamma"]), 1.0) and
                np.allclose(np.asarray(inputs["beta"]), 0.0))
    key = "nc_plain" if plain_gb else "nc"
    if key not in _cache:
        _cache[key] = _build(apply_gamma_beta=not plain_gb)
    nc = _cache[key]
    from concourse.bass_utils import run_bass_kernel_spmd

    names = ["Wq", "bq", "Wk", "bk", "Wv", "bv", "Wo", "bo", "gamma", "beta"]
    shared = {n: np.ascontiguousarray(np.asarray(inputs[n], dtype=np.float32))
              for n in names}
    x = np.asarray(inputs["x"], dtype=np.float32)
    in_maps = [dict(shared, x=np.ascontiguousarray(x[b])) for b in range(N_CORES)]
    res = run_bass_kernel_spmd(nc, in_maps, core_ids=list(range(N_CORES)))
    out = np.stack([res.results[i]["out"] for i in range(N_CORES)], axis=0)
    return out.astype(np.float32)

